# revision 1
# baseline (speedup 1.0000x reference)
"""Trainium2 Bass kernel for nn_MultiHeadAttention (B=8, S=1024, D=1024, H=16).

Sharding: data-parallel over batch — 8 NeuronCores, one batch element each;
weights replicated. No collectives needed.

Per-core plan (all matmul contractions on the partition dim, float32r PE
datapath = full rate at N=512, ~1e-4 rel err):

  phase A: PE-transpose x3/x1/x2 128x128 blocks (identity matmul) into
           x^T layouts; project v = (x3T as lhsT) @ Wk (natural [S, D]),
           interleaved with the x1/x2 transposes.  The bk bias rides as a
           K=1 accumulating matmul.  The key/pad mask is folded into an
           augmented value matrix vaug = [m * v | m] so masking AND the
           softmax denominator ride the PV matmul for free
           (P*m @ v == P @ (m*v), denom = P @ m).
  phase B: per head-pair p: q^T/k^T projections for pair p+1 are emitted
           as generators interleaved into pair p's attention loop (PE
           slack absorbs them, ACT stays saturated).  bq/bk biases fold
           into the PSUM->SBUF copy as per-partition tensor_scalar adds
           (no bias matmuls).  Scores S^T[k,q] = kT-slice^T @ qT-slice
           (K=64, the two heads auto-row-tile into PE row-groups via
           base_partition 0/64 => concurrent on HW); P^T = exp(S^T/8)
           via one [128,1024] ACT op per k-tile straight out of PSUM (no
           max-subtraction: |scores/8| <= ~7 is fp32-safe); O^T_aug[65,
           q] += vaug-slice^T @ P^T accumulated over k-tiles (row 64 =
           softmax denominator); epilogue per (pair, chunk): O rows
           stashed to SBUF to free the banks, 1/denom via DVE
           reciprocal_approx_fast (single custom op, ~51 ULP; keeps the
           epilogue off ACT, which the exp stream saturates on HW; plain
           DVE reciprocal is 8 cyc/elem, ACT Reciprocal/Rsqrt banned),
           partition-broadcast via the gpsimd ucode (SBUF->SBUF, no DRAM
           roundtrip), one [64,1024]-wide normalize pass into oT.  Exp
           and Ln are pinned to one activation-table set; the whole
           kernel uses only Exp/Ln/Identity so the table loads once.
  phase 3: out = (oT as lhsT) @ Wo; the bo bias (K=1 ones matmul) and
           the x1 residual (f32r identity matmul) ride the same PSUM
           accumulation, so LayerNorm stats read PSUM directly
           (bn_stats/bn_aggr on DVE); rstd = exp(-0.5*ln(var+eps)) stays
           in the pinned table set; the normalize (z-mu)*rstd runs on
           ACT as Identity(z*rstd + (-mu*rstd)) with per-partition
           scale/bias APs; gamma/beta halves split between DVE and
           gpsimd so no single engine owns the tail; wo row-block 0 is
           prefetched at kernel start (the main wo staging can only
           start once qkv SBUF frees).

TimelineSim: 315.9us (PE busy 246.9, ACT 209.3, DVE 93.9); see sim2.py /
simgaps.py.  HW device time tracked via the rep-slope method (kvar2.py
nrep): the axon-tunnel dispatch overhead ~4-5ms/call dominates the wall
metric with +-0.4ms session noise, so per-body device time is the
optimization target.  Measured correct on HW at rel err 1.9e-04.
"""
import sys

if "/opt/trn_rl_repo" not in sys.path:
    sys.path.insert(0, "/opt/trn_rl_repo")

import numpy as np

B, S, D, H = 8, 1024, 1024, 16
DK = D // H          # 64
NP = H // 2          # 8 head pairs
ST = S // 128        # 8 s-tiles (also k-tiles)
DT = D // 128        # 8 d-tiles
NC = S // 512        # 2 chunks of 512
VW = DK + 1          # 65: augmented head width
EPS = 1e-5

_BUILT = None


def _build():
    import concourse.bass as bass  # noqa: F401
    import concourse.tile as tile
    from concourse import bacc, mybir
    from concourse.masks import make_identity

    # Keep Exp and Ln in one activation-table set: remove them from every
    # other set (set order/indices preserved) so the table-load pass resolves
    # both to natural_log_exp_and_others instead of thrashing 33 reloads.
    AFt = mybir.ActivationFunctionType
    if not getattr(bacc, "_mha_act_tables_patched", False):
        orig_gat = bacc.get_activation_tables

        def _patched_gat(arch):
            t = dict(orig_gat(arch))
            for name, fns in t.items():
                if name != "natural_log_exp_and_others":
                    t[name] = {f for f in fns if f not in (AFt.Exp, AFt.Ln)}
            return t

        bacc.get_activation_tables = _patched_gat
        bacc._mha_act_tables_patched = True

    f32 = mybir.dt.float32
    f32r = mybir.dt.float32r
    AF = mybir.ActivationFunctionType

    nc = bacc.Bacc("TRN2", target_bir_lowering=False, debug=False, num_devices=B)

    # x inputs typed f32r: transposes run the 1.5-cyc/row f32r PE path and
    # the phase-3 residual identity-matmul can DMA straight from x1
    x1_ap = nc.dram_tensor("x1", [S, D], f32r, kind="ExternalInput").ap()
    x2_ap = nc.dram_tensor("x2", [S, D], f32r, kind="ExternalInput").ap()
    x3_ap = nc.dram_tensor("x3", [S, D], f32r, kind="ExternalInput").ap()
    wq_ap = nc.dram_tensor("wq", [D, D], f32r, kind="ExternalInput").ap()
    wk_ap = nc.dram_tensor("wk", [D, D], f32r, kind="ExternalInput").ap()
    wo_ap = nc.dram_tensor("wo", [D, D], f32r, kind="ExternalInput").ap()
    bq_ap = nc.dram_tensor("bq", [1, D], f32r, kind="ExternalInput").ap()
    bk_ap = nc.dram_tensor("bk", [1, D], f32r, kind="ExternalInput").ap()
    bo_ap = nc.dram_tensor("bo", [1, D], f32r, kind="ExternalInput").ap()
    gamma_ap = nc.dram_tensor("gamma", [1, D], f32, kind="ExternalInput").ap()
    beta_ap = nc.dram_tensor("beta", [1, D], f32, kind="ExternalInput").ap()
    mf_ap = nc.dram_tensor("mf", [S, 1], f32, kind="ExternalInput").ap()
    y_ap = nc.dram_tensor("y", [S, D], f32, kind="ExternalOutput").ap()

    with tile.TileContext(nc) as tc:
        with tc.tile_pool(name="persist", bufs=1) as persist:
            smalls = persist.tile([128, 512], f32)
            ident = smalls[:, 0:128]
            ones_p = smalls[:, 128:144]      # [128, 16] of ones
            eps_t = smalls[:, 144:145]
            m_sb = smalls[:, 145:153]        # [128, ST] mask per k-tile
            make_identity(nc, ident)
            nc.vector.memset(ones_p, 1.0)
            nc.vector.memset(eps_t, EPS)
            nc.gpsimd.dma_start(m_sb, mf_ap.rearrange("(t p) o -> p (t o)",
                                                       p=128))
            ones_f = persist.tile([1, 512], f32)
            nc.vector.memset(ones_f[:], 1.0)
            ones_r = persist.tile([1, 512], f32r)
            nc.vector.tensor_copy(ones_r[:], ones_f[:])
            ident_r = persist.tile([128, 128], f32r)
            nc.vector.tensor_copy(ident_r[:], ident)
            bk_sb = persist.tile([1, D], f32r)
            nc.gpsimd.dma_start(bk_sb[:], bk_ap[:])
            # bq/bk transposed to per-partition columns: bT[:, p] = b[p*128:...]
            bqT = persist.tile([128, DT], f32)
            nc.gpsimd.dma_start(
                bqT[:], bq_ap.bitcast(f32).rearrange("o (di p) -> p (o di)",
                                                     p=128))
            bkT = persist.tile([128, DT], f32)
            nc.gpsimd.dma_start(
                bkT[:], bk_ap.bitcast(f32).rearrange("o (di p) -> p (o di)",
                                                     p=128))
            # first Wo row-block prefetched at kernel start: the main wo_sb
            # staging can only DMA after qkv_pool's space frees, which would
            # stall phase 3's first matmuls
            wo0 = persist.tile([128, D], f32r)
            nc.gpsimd.dma_start(wo0[:], wo_ap[0:128, :])

            # oT outlives the phase-1/2 tensors: allocate below them
            oT_pool = tc.alloc_tile_pool(name="oTp", bufs=1)
            oT = oT_pool.tile([128, DT * S], f32r)
            # live through phases 1-2, released before phase 3
            qkv_pool = tc.alloc_tile_pool(name="qkv", bufs=1)
            x1T = qkv_pool.tile([128, DT * S], f32r)
            x2T = qkv_pool.tile([128, DT * S], f32r)
            vaug = qkv_pool.tile([128, ST * H * VW], f32r)  # k-tile t at t*H*VW

            # ------- phase A: transposes; v-projection (mask-augmented) -------
            with tc.tile_pool(name="pA_x3", bufs=1) as x3_pool, \
                 tc.tile_pool(name="pA_w", bufs=6) as w_pool, \
                 tc.tile_pool(name="pA_stage", bufs=10) as stage:

                pA_ps = tc.alloc_tile_pool(name="pA_ps", bufs=8, space="PSUM")

                def transpose_in(x_ap, xT):
                    # xT layout [128, DT*S]: d-tile dt at cols [dt*S + s]
                    xT3 = xT[:].rearrange("p (d s) -> p d s", s=S)
                    for st in range(ST):
                        for half in range(2):
                            xs = stage.tile([128, 512], f32r, name="xs", tag="xs")
                            nc.sync.dma_start(
                                xs[:], x_ap[st * 128:(st + 1) * 128,
                                            half * 512:(half + 1) * 512])
                            tp = pA_ps.tile([128, 512], f32r, name="tp", tag="ps512")
                            for j in range(4):
                                nc.tensor.transpose(
                                    tp[:, j * 128:(j + 1) * 128],
                                    xs[:, j * 128:(j + 1) * 128], ident_r[:])
                            dst = xT3[:, half * 4:half * 4 + 4,
                                      st * 128:(st + 1) * 128]
                            nc.scalar.copy(dst, tp[:].rearrange(
                                "p (b c) -> p b c", b=4))

                def v_proj_half(x3T, c):
                    # v natural [S, D] + augmentation with the mask
                    pss = [pA_ps.tile([128, 512], f32, name=f"vp{i}",
                                      tag="ps512") for i in range(ST)]
                    for di in range(DT):
                        wd = w_pool.tile([128, 512], f32r,
                                         name="wdv", tag="wd")
                        nc.sync.dma_start(
                            wd[:], wk_ap[di * 128:(di + 1) * 128,
                                         c * 512:(c + 1) * 512])
                        for st in range(ST):
                            nc.tensor.matmul(
                                pss[st][:],
                                x3T[:, di * S + st * 128:
                                    di * S + (st + 1) * 128],
                                wd[:], start=(di == 0), stop=False)
                    for st in range(ST):
                        nc.tensor.matmul(
                            pss[st][:], ones_r[:, 0:128],
                            bk_sb[:, c * 512:(c + 1) * 512],
                            start=False, stop=True)
                        va = vaug[:, st * H * VW:(st + 1) * H * VW].rearrange(
                            "p (h e) -> p h e", e=VW)
                        nc.vector.tensor_scalar_mul(
                            va[:, 8 * c:8 * (c + 1), 0:DK],
                            pss[st][:].rearrange("p (h e) -> p h e", e=DK),
                            m_sb[:, st:st + 1])
                        if c == 0:
                            nc.vector.tensor_scalar_mul(
                                va[:, :, DK:VW],
                                ones_p.rearrange("p (h e) -> p h e", e=1),
                                m_sb[:, st:st + 1])

                x3T = x3_pool.tile([128, DT * S], f32r)
                transpose_in(x3_ap, x3T)
                v_proj_half(x3T, 0)
                transpose_in(x1_ap, x1T)
                v_proj_half(x3T, 1)
                transpose_in(x2_ap, x2T)
                pA_ps.release()

            # --- phase B: per-pair q/k projection pipelined with attention ---
            with tc.tile_pool(name="pB_qk", bufs=2) as qk_pool, \
                 tc.tile_pool(name="pB_w", bufs=8) as w2_pool, \
                 tc.tile_pool(name="pB_P", bufs=4) as P_pool, \
                 tc.tile_pool(name="pB_scr", bufs=2) as scr_pool, \
                 tc.tile_pool(name="pB_pps", bufs=2, space="PSUM") as proj_ps, \
                 tc.tile_pool(name="pB_sps", bufs=2, space="PSUM") as s_ps, \
                 tc.tile_pool(name="pB_ops", bufs=2, space="PSUM") as o_ps:

                def proj_pair_gen(p, w_ap_, bT, xT, out):
                    # out[r, s] = sum_di (W[di, p-block] as lhsT) @ xT[di] + b
                    # generator: yields after each di so the caller can
                    # interleave these into the attention PE stream.
                    # di-outer: each weight tile is DMA'd once and feeds both
                    # q-chunks back-to-back (stationary reuse on the PE).
                    pps = [proj_ps.tile([128, 512], f32, name=f"pp{c}",
                                        tag="pp") for c in range(NC)]
                    for di in range(DT):
                        wd = w2_pool.tile([128, 128], f32r,
                                          name="wd2", tag="wd2")
                        nc.sync.dma_start(
                            wd[:], w_ap_[di * 128:(di + 1) * 128,
                                         p * 128:(p + 1) * 128])
                        for c in range(NC):
                            nc.tensor.matmul(
                                pps[c][:], wd[:],
                                xT[:, di * S + c * 512:di * S + (c + 1) * 512],
                                start=(di == 0), stop=(di == DT - 1))
                        yield
                    for c in range(NC):
                        # bias folded into the PSUM->SBUF copy (per-partition
                        # scalar add); no bias matmul needed
                        nc.vector.tensor_scalar(
                            out[:, c * 512:(c + 1) * 512], pps[c][:],
                            bT[:, p:p + 1], None,
                            op0=mybir.AluOpType.add)
                        yield

                def proj_pair(p):
                    q_t = qk_pool.tile([128, S], f32r, name=f"q{p}", tag="q")
                    k_t = qk_pool.tile([128, S], f32r, name=f"k{p}", tag="k")
                    gq = proj_pair_gen(p, wq_ap, bqT, x1T, q_t)
                    gk = proj_pair_gen(p, wk_ap, bkT, x2T, k_t)
                    return q_t, k_t, gq, gk

                def drain_gen(g, n=1000):
                    for _ in range(n):
                        try:
                            next(g)
                        except StopIteration:
                            return

                qTp, kTp, gq, gk = proj_pair(0)
                drain_gen(gq)
                drain_gen(gk)
                for p in range(NP):
                    # next pair's projections, interleaved into this pair's
                    # attention loop (PE slack absorbs them; ACT stays hot)
                    if p + 1 < NP:
                        qTn, kTn, gq, gk = proj_pair(p + 1)
                    else:
                        qTn = kTn = gq = gk = None
                    for c in range(NC):
                        oaugA = o_ps.tile([VW, 512], f32, name="oaugA", tag="oaug")
                        oaugB = o_ps.tile([VW, 512], f32, name="oaugB", tag="oaug")
                        for kt in range(ST):
                            sc = s_ps.tile([128, 1024], f32, name="sc", tag="sc")
                            nc.tensor.matmul(
                                sc[:, 0:512],
                                kTp[0:64, kt * 128:(kt + 1) * 128],
                                qTp[0:64, c * 512:(c + 1) * 512],
                                start=True, stop=True)
                            nc.tensor.matmul(
                                sc[:, 512:1024],
                                kTp[64:128, kt * 128:(kt + 1) * 128],
                                qTp[64:128, c * 512:(c + 1) * 512],
                                start=True, stop=True)
                            Pt = P_pool.tile([128, 1024], f32r, name="Pt", tag="Pt")
                            nc.scalar.activation(Pt[:], sc[:], AF.Exp,
                                                 scale=1.0 / float(np.sqrt(DK)))
                            base = kt * H * VW
                            nc.tensor.matmul(
                                oaugA[:],
                                vaug[:, base + 2 * p * VW:base + (2 * p + 1) * VW],
                                Pt[:, 0:512],
                                start=(kt == 0), stop=(kt == ST - 1))
                            nc.tensor.matmul(
                                oaugB[:],
                                vaug[:, base + (2 * p + 1) * VW:
                                     base + (2 * p + 2) * VW],
                                Pt[:, 512:1024],
                                start=(kt == 0), stop=(kt == ST - 1))
                            if gq is not None:
                                n = 1 if (kt % 2 == 0 or
                                          (c == 1 and kt in (1, 3))) else 0
                                drain_gen(gq, n)
                                drain_gen(gk, n)
                        # epilogue, both heads batched: stash O rows to free
                        # the banks, ln both denominators, one exp + one
                        # partition-broadcast for the pair
                        stash = scr_pool.tile([64, 1024], f32,
                                              name="stash", tag="stash")
                        nc.vector.tensor_copy(stash[:, 0:512], oaugA[0:64, :])
                        nc.vector.tensor_copy(stash[:, 512:1024], oaugB[0:64, :])
                        # 1/denominator on DVE (reciprocal_approx_fast is a
                        # single custom op, ~51 ULP): keeps the whole epilogue
                        # off ACT, which the exp stream saturates on HW
                        rec = scr_pool.tile([1, 1024], f32, name="rec", tag="rec")
                        nc.vector.tensor_copy(rec[:, 0:512], oaugA[64:65, :])
                        nc.vector.tensor_copy(rec[:, 512:1024], oaugB[64:65, :])
                        nc.vector.reciprocal_approx_fast(rec[:], rec[:])
                        rbc = scr_pool.tile([64, 1024], f32, name="rbc", tag="rbc")
                        nc.gpsimd.partition_broadcast(rbc[:], rec[:])
                        for h_loc in range(2):
                            nc.vector.tensor_mul(
                                oT[h_loc * 64:(h_loc + 1) * 64,
                                   p * S + c * 512:p * S + (c + 1) * 512],
                                stash[:, h_loc * 512:(h_loc + 1) * 512],
                                rbc[:, h_loc * 512:(h_loc + 1) * 512])
                    if gq is not None:
                        drain_gen(gq)
                        drain_gen(gk)
                        qTp, kTp = qTn, kTn
            qkv_pool.release()

            # ---------------- phase 3: out-proj + residual + LayerNorm --------
            with tc.tile_pool(name="p3_w", bufs=1) as w3_pool, \
                 tc.tile_pool(name="p3_stage", bufs=3) as stage3, \
                 tc.tile_pool(name="p3_t", bufs=4) as t_pool, \
                 tc.tile_pool(name="p3_ln", bufs=8) as ln_pool, \
                 tc.tile_pool(name="p3_ps", bufs=4, space="PSUM") as ps3:
                wo_sb = w3_pool.tile([128, (DT - 1) * D], f32r)
                for dt in range(1, DT):
                    nc.sync.dma_start(wo_sb[:, (dt - 1) * D:dt * D],
                                      wo_ap[dt * 128:(dt + 1) * 128, :])
                bo_sb = w3_pool.tile([1, D], f32r)
                nc.sync.dma_start(bo_sb[:], bo_ap[:])
                gamma_bc = w3_pool.tile([128, D], f32)
                nc.gpsimd.dma_start(gamma_bc[:], gamma_ap.partition_broadcast(128))
                beta_bc = w3_pool.tile([128, D], f32)
                nc.gpsimd.dma_start(beta_bc[:], beta_ap.partition_broadcast(128))
                for qt in range(ST):
                    ps = ps3.tile([128, 1024], f32, name="ps", tag="ps3")
                    xres = stage3.tile([128, D], f32r, name="xres", tag="xres")
                    nc.sync.dma_start(xres[:],
                                      x1_ap[qt * 128:(qt + 1) * 128, :])
                    for di in range(DT):
                        wsrc = (wo0[:] if di == 0 else
                                wo_sb[:, (di - 1) * D:di * D])
                        for c in range(NC):
                            nc.tensor.matmul(
                                ps[:, c * 512:(c + 1) * 512],
                                oT[:, di * S + qt * 128:di * S + (qt + 1) * 128],
                                wsrc[:, c * 512:(c + 1) * 512],
                                start=(di == 0), stop=False)
                    # bo bias and the x1 residual ride the accumulation as
                    # K=1 / identity matmuls (keeps the whole z off DVE)
                    for c in range(NC):
                        nc.tensor.matmul(
                            ps[:, c * 512:(c + 1) * 512], ones_r[:, 0:128],
                            bo_sb[:, c * 512:(c + 1) * 512],
                            start=False, stop=False)
                        nc.tensor.matmul(
                            ps[:, c * 512:(c + 1) * 512], ident_r[:],
                            xres[:, c * 512:(c + 1) * 512],
                            start=False, stop=True)
                    stats = ln_pool.tile([128, NC, 6], f32, name="stats", tag="st")
                    for c in range(NC):
                        nc.vector.bn_stats(stats[:, c, :],
                                           ps[:, c * 512:(c + 1) * 512])
                    mv = ln_pool.tile([128, 2], f32, name="mv", tag="mv")
                    nc.vector.bn_aggr(mv[:], stats[:])
                    # rstd = exp(-0.5*ln(var+eps)): stays in the pinned
                    # exp/ln table set (a Sqrt would force a table reload)
                    rstd = ln_pool.tile([128, 1], f32, name="rstd", tag="rstd")
                    nc.scalar.activation(rstd[:], mv[:, 1:2], AF.Ln, bias=eps_t)
                    nc.scalar.activation(rstd[:], rstd[:], AF.Exp, scale=-0.5)
                    # normalize on ACT (idle in phase 3): (z-mu)*rstd =
                    # Copy(z*rstd + (-mu*rstd)) with per-partition scale/bias
                    nmu = ln_pool.tile([128, 1], f32, name="nmu", tag="nmu")
                    nc.vector.tensor_scalar(
                        nmu[:], rstd[:], mv[:, 0:1], -1.0,
                        op0=mybir.AluOpType.mult,
                        op1=mybir.AluOpType.mult)
                    # per-half chains (DVE half 0, gpsimd half 1): half 0's
                    # store fires while half 1 is still in its affine
                    t2 = t_pool.tile([128, D], f32, name="t2", tag="t2")
                    t3 = t_pool.tile([128, D], f32, name="t3", tag="t3")
                    t4 = t_pool.tile([128, D], f32, name="t4", tag="t4")
                    for h, eng in ((0, nc.vector), (1, nc.gpsimd)):
                        cs = slice(h * 512, (h + 1) * 512)
                        nc.scalar.activation(t2[:, cs], ps[:, cs], AF.Identity,
                                             scale=rstd[:], bias=nmu[:])
                        eng.tensor_mul(t3[:, cs], t2[:, cs], gamma_bc[:, cs])
                        eng.tensor_add(t4[:, cs], t3[:, cs], beta_bc[:, cs])
                        nc.sync.dma_start(
                            y_ap[qt * 128:(qt + 1) * 128, cs], t4[:, cs])
            oT_pool.release()

    nc.compile()
    return nc


def _get_built():
    global _BUILT
    if _BUILT is None:
        _BUILT = _build()
    return _BUILT


def kernel(x1, x2, x3, mask, Wq, bq, Wk, bk, Wo, bo, gamma, beta):
    from concourse import bass_utils

    nc = _get_built()
    x1 = np.ascontiguousarray(np.asarray(x1, np.float32))
    x2 = np.ascontiguousarray(np.asarray(x2, np.float32))
    x3 = np.ascontiguousarray(np.asarray(x3, np.float32))
    mf = (np.asarray(mask) != 0).astype(np.float32)          # [B, 1, S]
    shared = {
        "wq": np.ascontiguousarray(np.asarray(Wq, np.float32)),
        "wk": np.ascontiguousarray(np.asarray(Wk, np.float32)),
        "wo": np.ascontiguousarray(np.asarray(Wo, np.float32)),
        "bq": np.asarray(bq, np.float32).reshape(1, D),
        "bk": np.asarray(bk, np.float32).reshape(1, D),
        "bo": np.asarray(bo, np.float32).reshape(1, D),
        "gamma": np.asarray(gamma, np.float32).reshape(1, D),
        "beta": np.asarray(beta, np.float32).reshape(1, D),
    }
    in_maps = []
    for b in range(B):
        m = dict(shared)
        m["x1"] = x1[b]
        m["x2"] = x2[b]
        m["x3"] = x3[b]
        m["mf"] = np.ascontiguousarray(mf[b, 0, :].reshape(S, 1))
        in_maps.append(m)
    res = bass_utils.run_bass_kernel_spmd(nc, in_maps, core_ids=list(range(B)))
    return np.stack([res.results[b]["y"] for b in range(B)])



# revision 4
# speedup vs baseline: 9.8422x; 9.8422x over previous
"""Trainium2 Bass kernel for nn_MultiHeadAttention (B=8, S=1024, D=1024, H=16).

Sharding: data-parallel over batch — 8 NeuronCores, one batch element each;
weights replicated. No collectives needed.

IO packing: the per-call dispatch overhead through the axon tunnel scales
with the number of IO tensors (~50us/tensor/call) and IO bytes, and
dominates the pipelined per-call wall time (device body ~0.35ms overlaps
the dispatch pipeline entirely).  All 13 inputs are therefore packed into
ONE [6150, 1024] f32 DRAM tensor per core (x1|x2|x3|wq|wk|wo rows 0-6143,
then mf/bq/bk/bo/gamma/beta one row each), cutting per-call tensor count
from 15 to 3.

Per-core compute plan (all matmul contractions on the partition dim,
float32r PE datapath = full rate at N=512, ~1e-4 rel err):

  phase A: PE-transpose x3/x1/x2 128x128 blocks (identity matmul) into
           x^T layouts; project v = (x3T as lhsT) @ Wk (natural [S, D]),
           interleaved with the x1/x2 transposes.  The bk bias rides as a
           K=1 accumulating matmul.  The key/pad mask is folded into an
           augmented value matrix vaug = [m * v | m] so masking AND the
           softmax denominator ride the PV matmul for free
           (P*m @ v == P @ (m*v), denom = P @ m).
  phase B: per head-pair p: q^T/k^T projections for pair p+1 are emitted
           as generators interleaved into pair p's attention loop (PE
           slack absorbs them, ACT stays saturated).  bq/bk biases fold
           into the PSUM->SBUF copy as per-partition tensor_scalar adds
           (no bias matmuls).  Scores S^T[k,q] = kT-slice^T @ qT-slice
           (K=64, the two heads auto-row-tile into PE row-groups via
           base_partition 0/64 => concurrent on HW); P^T = exp(S^T/8)
           via one [128,1024] ACT op per k-tile straight out of PSUM (no
           max-subtraction: |scores/8| <= ~7 is fp32-safe); O^T_aug[65,
           q] += vaug-slice^T @ P^T accumulated over k-tiles (row 64 =
           softmax denominator); epilogue per (pair, chunk): O rows
           stashed to SBUF to free the banks, 1/denom via DVE
           reciprocal_approx_fast (single custom op, ~51 ULP; keeps the
           epilogue off ACT, which the exp stream saturates on HW; plain
           DVE reciprocal is 8 cyc/elem, ACT Reciprocal/Rsqrt banned),
           partition-broadcast via the gpsimd ucode (SBUF->SBUF, no DRAM
           roundtrip), one [64,1024]-wide normalize pass into oT.  Exp
           and Ln are pinned to one activation-table set; the whole
           kernel uses only Exp/Ln/Identity so the table loads once.
  phase 3: out = (oT as lhsT) @ Wo; the bo bias (K=1 ones matmul) and
           the x1 residual (f32r identity matmul) ride the same PSUM
           accumulation, so LayerNorm stats read PSUM directly
           (bn_stats/bn_aggr on DVE); rstd = exp(-0.5*ln(var+eps)) stays
           in the pinned table set; the normalize (z-mu)*rstd runs on
           ACT as Identity(z*rstd + (-mu*rstd)) with per-partition
           scale/bias APs; gamma/beta halves split between DVE and
           gpsimd so no single engine owns the tail; wo row-block 0 is
           prefetched at kernel start (the main wo staging can only
           start once qkv SBUF frees).
"""
import sys

if "/opt/trn_rl_repo" not in sys.path:
    sys.path.insert(0, "/opt/trn_rl_repo")

import numpy as np

B, S, D, H = 8, 1024, 1024, 16
DK = D // H          # 64
NP = H // 2          # 8 head pairs
ST = S // 128        # 8 s-tiles (also k-tiles)
DT = D // 128        # 8 d-tiles
NC = S // 512        # 2 chunks of 512
VW = DK + 1          # 65: augmented head width
EPS = 1e-5

# packed input layout: row offsets into xin [6150, 1024]
R_X1, R_X2, R_X3 = 0, S, 2 * S
R_WQ, R_WK, R_WO = 3 * S, 3 * S + D, 3 * S + 2 * D
R_MF = 3 * S + 3 * D          # 6144: mask row [1, 1024]
R_BQ, R_BK, R_BO = R_MF + 1, R_MF + 2, R_MF + 3
R_GAMMA, R_BETA = R_MF + 4, R_MF + 5
N_ROWS = R_MF + 6             # 6150

_BUILT = None


def _build():
    import concourse.bass as bass  # noqa: F401
    import concourse.tile as tile
    from concourse import bacc, mybir
    from concourse.masks import make_identity

    # Keep Exp and Ln in one activation-table set: remove them from every
    # other set (set order/indices preserved) so the table-load pass resolves
    # both to natural_log_exp_and_others instead of thrashing 33 reloads.
    AFt = mybir.ActivationFunctionType
    if not getattr(bacc, "_mha_act_tables_patched", False):
        orig_gat = bacc.get_activation_tables

        def _patched_gat(arch):
            t = dict(orig_gat(arch))
            for name, fns in t.items():
                if name != "natural_log_exp_and_others":
                    t[name] = {f for f in fns if f not in (AFt.Exp, AFt.Ln)}
            return t

        bacc.get_activation_tables = _patched_gat
        bacc._mha_act_tables_patched = True

    f32 = mybir.dt.float32
    f32r = mybir.dt.float32r
    AF = mybir.ActivationFunctionType

    nc = bacc.Bacc("TRN2", target_bir_lowering=False, debug=False, num_devices=B)

    # ONE packed input tensor; slices bitcast/viewed per use. f32r typing:
    # transposes run the 1.5-cyc/row f32r PE path and the phase-3 residual
    # identity-matmul can DMA straight from the x1 slice.
    xin = nc.dram_tensor("xin", [N_ROWS, D], f32r, kind="ExternalInput").ap()
    x1_ap = xin[R_X1:R_X1 + S, :]
    x2_ap = xin[R_X2:R_X2 + S, :]
    x3_ap = xin[R_X3:R_X3 + S, :]
    wq_ap = xin[R_WQ:R_WQ + D, :]
    wk_ap = xin[R_WK:R_WK + D, :]
    wo_ap = xin[R_WO:R_WO + D, :]
    bq_ap = xin[R_BQ:R_BQ + 1, :]
    bk_ap = xin[R_BK:R_BK + 1, :]
    bo_ap = xin[R_BO:R_BO + 1, :]
    gamma_ap = xin[R_GAMMA:R_GAMMA + 1, :].bitcast(f32)
    beta_ap = xin[R_BETA:R_BETA + 1, :].bitcast(f32)
    mf_ap = xin[R_MF:R_MF + 1, :].bitcast(f32)   # [1, 1024] mask row
    y_ap = nc.dram_tensor("y", [S, D], f32, kind="ExternalOutput").ap()

    with tile.TileContext(nc) as tc:
        with tc.tile_pool(name="persist", bufs=1) as persist:
            smalls = persist.tile([128, 512], f32)
            ident = smalls[:, 0:128]
            ones_p = smalls[:, 128:144]      # [128, 16] of ones
            eps_t = smalls[:, 144:145]
            m_sb = smalls[:, 145:153]        # [128, ST] mask per k-tile
            make_identity(nc, ident)
            nc.vector.memset(ones_p, 1.0)
            nc.vector.memset(eps_t, EPS)
            nc.gpsimd.dma_start(m_sb, mf_ap.rearrange("o (t p) -> p (t o)",
                                                      p=128))
            ones_f = persist.tile([1, 512], f32)
            nc.vector.memset(ones_f[:], 1.0)
            ones_r = persist.tile([1, 512], f32r)
            nc.vector.tensor_copy(ones_r[:], ones_f[:])
            ident_r = persist.tile([128, 128], f32r)
            nc.vector.tensor_copy(ident_r[:], ident)
            bk_sb = persist.tile([1, D], f32r)
            nc.gpsimd.dma_start(bk_sb[:], bk_ap)
            # bq/bk transposed to per-partition columns: bT[:, p] = b[p*128:...]
            bqT = persist.tile([128, DT], f32)
            nc.gpsimd.dma_start(
                bqT[:], bq_ap.bitcast(f32).rearrange("o (di p) -> p (o di)",
                                                     p=128))
            bkT = persist.tile([128, DT], f32)
            nc.gpsimd.dma_start(
                bkT[:], bk_ap.bitcast(f32).rearrange("o (di p) -> p (o di)",
                                                     p=128))
            # first Wo row-block prefetched at kernel start: the main wo_sb
            # staging can only DMA after qkv_pool's space frees, which would
            # stall phase 3's first matmuls
            wo0 = persist.tile([128, D], f32r)
            nc.gpsimd.dma_start(wo0[:], wo_ap[0:128, :])

            # oT outlives the phase-1/2 tensors: allocate below them
            oT_pool = tc.alloc_tile_pool(name="oTp", bufs=1)
            oT = oT_pool.tile([128, DT * S], f32r)
            # live through phases 1-2, released before phase 3
            qkv_pool = tc.alloc_tile_pool(name="qkv", bufs=1)
            x1T = qkv_pool.tile([128, DT * S], f32r)
            x2T = qkv_pool.tile([128, DT * S], f32r)
            vaug = qkv_pool.tile([128, ST * H * VW], f32r)  # k-tile t at t*H*VW

            # ------- phase A: transposes; v-projection (mask-augmented) -------
            with tc.tile_pool(name="pA_x3", bufs=1) as x3_pool, \
                 tc.tile_pool(name="pA_w", bufs=6) as w_pool, \
                 tc.tile_pool(name="pA_stage", bufs=10) as stage:

                pA_ps = tc.alloc_tile_pool(name="pA_ps", bufs=8, space="PSUM")

                def transpose_in(x_ap, xT):
                    # xT layout [128, DT*S]: d-tile dt at cols [dt*S + s]
                    xT3 = xT[:].rearrange("p (d s) -> p d s", s=S)
                    for st in range(ST):
                        for half in range(2):
                            xs = stage.tile([128, 512], f32r, name="xs", tag="xs")
                            nc.sync.dma_start(
                                xs[:], x_ap[st * 128:(st + 1) * 128,
                                            half * 512:(half + 1) * 512])
                            tp = pA_ps.tile([128, 512], f32r, name="tp", tag="ps512")
                            for j in range(4):
                                nc.tensor.transpose(
                                    tp[:, j * 128:(j + 1) * 128],
                                    xs[:, j * 128:(j + 1) * 128], ident_r[:])
                            dst = xT3[:, half * 4:half * 4 + 4,
                                      st * 128:(st + 1) * 128]
                            nc.scalar.copy(dst, tp[:].rearrange(
                                "p (b c) -> p b c", b=4))

                def v_proj_half(x3T, c):
                    # v natural [S, D] + augmentation with the mask
                    pss = [pA_ps.tile([128, 512], f32, name=f"vp{i}",
                                      tag="ps512") for i in range(ST)]
                    for di in range(DT):
                        wd = w_pool.tile([128, 512], f32r,
                                         name="wdv", tag="wd")
                        nc.sync.dma_start(
                            wd[:], wk_ap[di * 128:(di + 1) * 128,
                                         c * 512:(c + 1) * 512])
                        for st in range(ST):
                            nc.tensor.matmul(
                                pss[st][:],
                                x3T[:, di * S + st * 128:
                                    di * S + (st + 1) * 128],
                                wd[:], start=(di == 0), stop=False)
                    for st in range(ST):
                        nc.tensor.matmul(
                            pss[st][:], ones_r[:, 0:128],
                            bk_sb[:, c * 512:(c + 1) * 512],
                            start=False, stop=True)
                        va = vaug[:, st * H * VW:(st + 1) * H * VW].rearrange(
                            "p (h e) -> p h e", e=VW)
                        nc.vector.tensor_scalar_mul(
                            va[:, 8 * c:8 * (c + 1), 0:DK],
                            pss[st][:].rearrange("p (h e) -> p h e", e=DK),
                            m_sb[:, st:st + 1])
                        if c == 0:
                            nc.vector.tensor_scalar_mul(
                                va[:, :, DK:VW],
                                ones_p.rearrange("p (h e) -> p h e", e=1),
                                m_sb[:, st:st + 1])

                x3T = x3_pool.tile([128, DT * S], f32r)
                transpose_in(x3_ap, x3T)
                v_proj_half(x3T, 0)
                transpose_in(x1_ap, x1T)
                v_proj_half(x3T, 1)
                transpose_in(x2_ap, x2T)
                pA_ps.release()

            # --- phase B: per-pair q/k projection pipelined with attention ---
            with tc.tile_pool(name="pB_qk", bufs=2) as qk_pool, \
                 tc.tile_pool(name="pB_w", bufs=8) as w2_pool, \
                 tc.tile_pool(name="pB_P", bufs=4) as P_pool, \
                 tc.tile_pool(name="pB_scr", bufs=2) as scr_pool, \
                 tc.tile_pool(name="pB_pps", bufs=2, space="PSUM") as proj_ps, \
                 tc.tile_pool(name="pB_sps", bufs=2, space="PSUM") as s_ps, \
                 tc.tile_pool(name="pB_ops", bufs=2, space="PSUM") as o_ps:

                def proj_pair_gen(p, w_ap_, bT, xT, out):
                    # out[r, s] = sum_di (W[di, p-block] as lhsT) @ xT[di] + b
                    # generator: yields after each di so the caller can
                    # interleave these into the attention PE stream.
                    # di-outer: each weight tile is DMA'd once and feeds both
                    # q-chunks back-to-back (stationary reuse on the PE).
                    pps = [proj_ps.tile([128, 512], f32, name=f"pp{c}",
                                        tag="pp") for c in range(NC)]
                    for di in range(DT):
                        wd = w2_pool.tile([128, 128], f32r,
                                          name="wd2", tag="wd2")
                        nc.sync.dma_start(
                            wd[:], w_ap_[di * 128:(di + 1) * 128,
                                         p * 128:(p + 1) * 128])
                        for c in range(NC):
                            nc.tensor.matmul(
                                pps[c][:], wd[:],
                                xT[:, di * S + c * 512:di * S + (c + 1) * 512],
                                start=(di == 0), stop=(di == DT - 1))
                        yield
                    for c in range(NC):
                        # bias folded into the PSUM->SBUF copy (per-partition
                        # scalar add); no bias matmul needed
                        nc.vector.tensor_scalar(
                            out[:, c * 512:(c + 1) * 512], pps[c][:],
                            bT[:, p:p + 1], None,
                            op0=mybir.AluOpType.add)
                        yield

                def proj_pair(p):
                    q_t = qk_pool.tile([128, S], f32r, name=f"q{p}", tag="q")
                    k_t = qk_pool.tile([128, S], f32r, name=f"k{p}", tag="k")
                    gq = proj_pair_gen(p, wq_ap, bqT, x1T, q_t)
                    gk = proj_pair_gen(p, wk_ap, bkT, x2T, k_t)
                    return q_t, k_t, gq, gk

                def drain_gen(g, n=1000):
                    for _ in range(n):
                        try:
                            next(g)
                        except StopIteration:
                            return

                qTp, kTp, gq, gk = proj_pair(0)
                drain_gen(gq)
                drain_gen(gk)
                for p in range(NP):
                    # next pair's projections, interleaved into this pair's
                    # attention loop (PE slack absorbs them; ACT stays hot)
                    if p + 1 < NP:
                        qTn, kTn, gq, gk = proj_pair(p + 1)
                    else:
                        qTn = kTn = gq = gk = None
                    for c in range(NC):
                        oaugA = o_ps.tile([VW, 512], f32, name="oaugA", tag="oaug")
                        oaugB = o_ps.tile([VW, 512], f32, name="oaugB", tag="oaug")
                        for kt in range(ST):
                            sc = s_ps.tile([128, 1024], f32, name="sc", tag="sc")
                            nc.tensor.matmul(
                                sc[:, 0:512],
                                kTp[0:64, kt * 128:(kt + 1) * 128],
                                qTp[0:64, c * 512:(c + 1) * 512],
                                start=True, stop=True)
                            nc.tensor.matmul(
                                sc[:, 512:1024],
                                kTp[64:128, kt * 128:(kt + 1) * 128],
                                qTp[64:128, c * 512:(c + 1) * 512],
                                start=True, stop=True)
                            Pt = P_pool.tile([128, 1024], f32r, name="Pt", tag="Pt")
                            nc.scalar.activation(Pt[:], sc[:], AF.Exp,
                                                 scale=1.0 / float(np.sqrt(DK)))
                            base = kt * H * VW
                            nc.tensor.matmul(
                                oaugA[:],
                                vaug[:, base + 2 * p * VW:base + (2 * p + 1) * VW],
                                Pt[:, 0:512],
                                start=(kt == 0), stop=(kt == ST - 1))
                            nc.tensor.matmul(
                                oaugB[:],
                                vaug[:, base + (2 * p + 1) * VW:
                                     base + (2 * p + 2) * VW],
                                Pt[:, 512:1024],
                                start=(kt == 0), stop=(kt == ST - 1))
                            if gq is not None:
                                n = 1 if (kt % 2 == 0 or
                                          (c == 1 and kt in (1, 3))) else 0
                                drain_gen(gq, n)
                                drain_gen(gk, n)
                        # epilogue, both heads batched: stash O rows to free
                        # the banks, ln both denominators, one exp + one
                        # partition-broadcast for the pair
                        stash = scr_pool.tile([64, 1024], f32,
                                              name="stash", tag="stash")
                        nc.vector.tensor_copy(stash[:, 0:512], oaugA[0:64, :])
                        nc.vector.tensor_copy(stash[:, 512:1024], oaugB[0:64, :])
                        # 1/denominator on DVE (reciprocal_approx_fast is a
                        # single custom op, ~51 ULP): keeps the whole epilogue
                        # off ACT, which the exp stream saturates on HW
                        rec = scr_pool.tile([1, 1024], f32, name="rec", tag="rec")
                        nc.vector.tensor_copy(rec[:, 0:512], oaugA[64:65, :])
                        nc.vector.tensor_copy(rec[:, 512:1024], oaugB[64:65, :])
                        nc.vector.reciprocal_approx_fast(rec[:], rec[:])
                        rbc = scr_pool.tile([64, 1024], f32, name="rbc", tag="rbc")
                        nc.gpsimd.partition_broadcast(rbc[:], rec[:])
                        for h_loc in range(2):
                            nc.vector.tensor_mul(
                                oT[h_loc * 64:(h_loc + 1) * 64,
                                   p * S + c * 512:p * S + (c + 1) * 512],
                                stash[:, h_loc * 512:(h_loc + 1) * 512],
                                rbc[:, h_loc * 512:(h_loc + 1) * 512])
                    if gq is not None:
                        drain_gen(gq)
                        drain_gen(gk)
                        qTp, kTp = qTn, kTn
            qkv_pool.release()

            # ---------------- phase 3: out-proj + residual + LayerNorm --------
            with tc.tile_pool(name="p3_w", bufs=1) as w3_pool, \
                 tc.tile_pool(name="p3_stage", bufs=3) as stage3, \
                 tc.tile_pool(name="p3_t", bufs=4) as t_pool, \
                 tc.tile_pool(name="p3_ln", bufs=8) as ln_pool, \
                 tc.tile_pool(name="p3_ps", bufs=4, space="PSUM") as ps3:
                wo_sb = w3_pool.tile([128, (DT - 1) * D], f32r)
                for dt in range(1, DT):
                    nc.sync.dma_start(wo_sb[:, (dt - 1) * D:dt * D],
                                      wo_ap[dt * 128:(dt + 1) * 128, :])
                bo_sb = w3_pool.tile([1, D], f32r)
                nc.sync.dma_start(bo_sb[:], bo_ap)
                gamma_bc = w3_pool.tile([128, D], f32)
                nc.gpsimd.dma_start(gamma_bc[:], gamma_ap.partition_broadcast(128))
                beta_bc = w3_pool.tile([128, D], f32)
                nc.gpsimd.dma_start(beta_bc[:], beta_ap.partition_broadcast(128))
                for qt in range(ST):
                    ps = ps3.tile([128, 1024], f32, name="ps", tag="ps3")
                    xres = stage3.tile([128, D], f32r, name="xres", tag="xres")
                    nc.sync.dma_start(xres[:],
                                      x1_ap[qt * 128:(qt + 1) * 128, :])
                    for di in range(DT):
                        wsrc = (wo0[:] if di == 0 else
                                wo_sb[:, (di - 1) * D:di * D])
                        for c in range(NC):
                            nc.tensor.matmul(
                                ps[:, c * 512:(c + 1) * 512],
                                oT[:, di * S + qt * 128:di * S + (qt + 1) * 128],
                                wsrc[:, c * 512:(c + 1) * 512],
                                start=(di == 0), stop=False)
                    # bo bias and the x1 residual ride the accumulation as
                    # K=1 / identity matmuls (keeps the whole z off DVE)
                    for c in range(NC):
                        nc.tensor.matmul(
                            ps[:, c * 512:(c + 1) * 512], ones_r[:, 0:128],
                            bo_sb[:, c * 512:(c + 1) * 512],
                            start=False, stop=False)
                        nc.tensor.matmul(
                            ps[:, c * 512:(c + 1) * 512], ident_r[:],
                            xres[:, c * 512:(c + 1) * 512],
                            start=False, stop=True)
                    stats = ln_pool.tile([128, NC, 6], f32, name="stats", tag="st")
                    for c in range(NC):
                        nc.vector.bn_stats(stats[:, c, :],
                                           ps[:, c * 512:(c + 1) * 512])
                    mv = ln_pool.tile([128, 2], f32, name="mv", tag="mv")
                    nc.vector.bn_aggr(mv[:], stats[:])
                    # rstd = exp(-0.5*ln(var+eps)): stays in the pinned
                    # exp/ln table set (a Sqrt would force a table reload)
                    rstd = ln_pool.tile([128, 1], f32, name="rstd", tag="rstd")
                    nc.scalar.activation(rstd[:], mv[:, 1:2], AF.Ln, bias=eps_t)
                    nc.scalar.activation(rstd[:], rstd[:], AF.Exp, scale=-0.5)
                    # normalize on ACT (idle in phase 3): (z-mu)*rstd =
                    # Copy(z*rstd + (-mu*rstd)) with per-partition scale/bias
                    nmu = ln_pool.tile([128, 1], f32, name="nmu", tag="nmu")
                    nc.vector.tensor_scalar(
                        nmu[:], rstd[:], mv[:, 0:1], -1.0,
                        op0=mybir.AluOpType.mult,
                        op1=mybir.AluOpType.mult)
                    # per-half chains (DVE half 0, gpsimd half 1): half 0's
                    # store fires while half 1 is still in its affine
                    t2 = t_pool.tile([128, D], f32, name="t2", tag="t2")
                    t3 = t_pool.tile([128, D], f32, name="t3", tag="t3")
                    t4 = t_pool.tile([128, D], f32, name="t4", tag="t4")
                    for h, eng in ((0, nc.vector), (1, nc.gpsimd)):
                        cs = slice(h * 512, (h + 1) * 512)
                        nc.scalar.activation(t2[:, cs], ps[:, cs], AF.Identity,
                                             scale=rstd[:], bias=nmu[:])
                        eng.tensor_mul(t3[:, cs], t2[:, cs], gamma_bc[:, cs])
                        eng.tensor_add(t4[:, cs], t3[:, cs], beta_bc[:, cs])
                        nc.sync.dma_start(
                            y_ap[qt * 128:(qt + 1) * 128, cs], t4[:, cs])
            oT_pool.release()

    nc.compile()
    return nc


def _get_built():
    global _BUILT
    if _BUILT is None:
        _BUILT = _build()
    return _BUILT


def pack_input(b, x1, x2, x3, mf, Wq, Wk, Wo, bq, bk, bo, gamma, beta):
    """Build the packed [6150, 1024] f32 input for batch element b."""
    xin = np.empty((N_ROWS, D), np.float32)
    xin[R_X1:R_X1 + S] = x1[b]
    xin[R_X2:R_X2 + S] = x2[b]
    xin[R_X3:R_X3 + S] = x3[b]
    xin[R_WQ:R_WQ + D] = Wq
    xin[R_WK:R_WK + D] = Wk
    xin[R_WO:R_WO + D] = Wo
    xin[R_MF] = mf[b, 0, :]
    xin[R_BQ] = bq
    xin[R_BK] = bk
    xin[R_BO] = bo
    xin[R_GAMMA] = gamma
    xin[R_BETA] = beta
    return xin


def kernel(x1, x2, x3, mask, Wq, bq, Wk, bk, Wo, bo, gamma, beta):
    from concourse import bass_utils

    nc = _get_built()
    x1 = np.asarray(x1, np.float32)
    x2 = np.asarray(x2, np.float32)
    x3 = np.asarray(x3, np.float32)
    mf = (np.asarray(mask) != 0).astype(np.float32)          # [B, 1, S]
    Wq = np.asarray(Wq, np.float32)
    Wk = np.asarray(Wk, np.float32)
    Wo = np.asarray(Wo, np.float32)
    bq = np.asarray(bq, np.float32).reshape(D)
    bk = np.asarray(bk, np.float32).reshape(D)
    bo = np.asarray(bo, np.float32).reshape(D)
    gamma = np.asarray(gamma, np.float32).reshape(D)
    beta = np.asarray(beta, np.float32).reshape(D)
    in_maps = [
        {"xin": pack_input(b, x1, x2, x3, mf, Wq, Wk, Wo,
                           bq, bk, bo, gamma, beta)}
        for b in range(B)
    ]
    res = bass_utils.run_bass_kernel_spmd(nc, in_maps, core_ids=list(range(B)))
    return np.stack([res.results[b]["y"] for b in range(B)])


# revision 15
# speedup vs baseline: 10.1174x; 1.0280x over previous
"""Trainium2 Bass kernel for nn_MultiHeadAttention (B=8, S=1024, D=1024, H=16).

Sharding: data-parallel over batch — 8 NeuronCores, one batch element each;
weights replicated. No collectives needed.

IO packing: the per-call dispatch overhead through the axon tunnel scales
with the number of IO tensors (~50us/tensor/call) and IO bytes, and
dominates the pipelined per-call wall time (device body ~0.35ms overlaps
the dispatch pipeline entirely).  All 13 inputs are therefore packed into
ONE [6150, 1024] f32 DRAM tensor per core (x1|x2|x3|wq|wk|wo rows 0-6143,
then mf/bq/bk/bo/gamma/beta one row each), cutting per-call tensor count
from 15 to 3.

Per-core compute plan (all matmul contractions on the partition dim).
The q/k/v projections run as fp8e4m3 DoubleRow matmuls (both operands
fp8, two 128-row d-tiles fused per instruction at 0.5 cyc/row => 4x
fewer PE cycles than f32r); x^T tiles are cast to fp8 in the
transpose PSUM->SBUF copy and weights on DVE after staging.  Scores,
PV, and the out-projection stay f32r (full rate at N=512).  Measured
rel err 9.5e-3 on HW (gate 2e-2; error dominated by fp8 q/k
quantization amplified through exp).  Paired 10-round A/B on HW:
-47 us/call median vs the f32r projections:

  phase A: PE-transpose x3/x1/x2 128x128 blocks (identity matmul) into
           x^T layouts; project v = (x3T as lhsT) @ Wk (natural [S, D]),
           interleaved with the x1/x2 transposes.  The bk bias rides as a
           K=1 accumulating matmul.  The key/pad mask is folded into an
           augmented value matrix vaug = [m * v | m] so masking AND the
           softmax denominator ride the PV matmul for free
           (P*m @ v == P @ (m*v), denom = P @ m).
  phase B: per head-pair p: q^T/k^T projections for pair p+1 are emitted
           as generators interleaved into pair p's attention loop (PE
           slack absorbs them, ACT stays saturated).  bq/bk biases fold
           into the PSUM->SBUF copy as per-partition tensor_scalar adds
           (no bias matmuls).  Scores S^T[k,q] = kT-slice^T @ qT-slice
           (K=64, the two heads auto-row-tile into PE row-groups via
           base_partition 0/64 => concurrent on HW); P^T = exp(S^T/8)
           via one [128,1024] ACT op per k-tile straight out of PSUM (no
           max-subtraction: |scores/8| <= ~7 is fp32-safe); O^T_aug[65,
           q] += vaug-slice^T @ P^T accumulated over k-tiles (row 64 =
           softmax denominator); epilogue per (pair, chunk): O rows
           stashed to SBUF to free the banks, 1/denom via DVE
           reciprocal_approx_fast (single custom op, ~51 ULP; keeps the
           epilogue off ACT, which the exp stream saturates on HW; plain
           DVE reciprocal is 8 cyc/elem, ACT Reciprocal/Rsqrt banned),
           partition-broadcast via the gpsimd ucode (SBUF->SBUF, no DRAM
           roundtrip), one [64,1024]-wide normalize pass into oT.  Exp
           and Ln are pinned to one activation-table set; the whole
           kernel uses only Exp/Ln/Identity so the table loads once.
  phase 3: out = (oT as lhsT) @ Wo; the bo bias (K=1 ones matmul) and
           the x1 residual (f32r identity matmul) ride the same PSUM
           accumulation, so LayerNorm stats read PSUM directly
           (bn_stats/bn_aggr on DVE); rstd = exp(-0.5*ln(var+eps)) stays
           in the pinned table set; the normalize (z-mu)*rstd runs on
           ACT as Identity(z*rstd + (-mu*rstd)) with per-partition
           scale/bias APs; gamma/beta halves split between DVE and
           gpsimd so no single engine owns the tail; wo row-block 0 is
           prefetched at kernel start (the main wo staging can only
           start once qkv SBUF frees).
"""
import sys

if "/opt/trn_rl_repo" not in sys.path:
    sys.path.insert(0, "/opt/trn_rl_repo")

import numpy as np

B, S, D, H = 8, 1024, 1024, 16
DK = D // H          # 64
NP = H // 2          # 8 head pairs
ST = S // 128        # 8 s-tiles (also k-tiles)
DT = D // 128        # 8 d-tiles
NC = S // 512        # 2 chunks of 512
VW = DK + 1          # 65: augmented head width
EPS = 1e-5

# packed input layout: row offsets into xin [6150, 1024]
R_X1, R_X2, R_X3 = 0, S, 2 * S
R_WQ, R_WK, R_WO = 3 * S, 3 * S + D, 3 * S + 2 * D
R_MF = 3 * S + 3 * D          # 6144: mask row [1, 1024]
R_BQ, R_BK, R_BO = R_MF + 1, R_MF + 2, R_MF + 3
R_GAMMA, R_BETA = R_MF + 4, R_MF + 5
N_ROWS = R_MF + 6             # 6150

_BUILT = None


def _build():
    import concourse.bass as bass  # noqa: F401
    import concourse.tile as tile
    from concourse import bacc, mybir
    from concourse.masks import make_identity

    # Keep Exp and Ln in one activation-table set: remove them from every
    # other set (set order/indices preserved) so the table-load pass resolves
    # both to natural_log_exp_and_others instead of thrashing 33 reloads.
    AFt = mybir.ActivationFunctionType
    if not getattr(bacc, "_mha_act_tables_patched", False):
        orig_gat = bacc.get_activation_tables

        def _patched_gat(arch):
            t = dict(orig_gat(arch))
            for name, fns in t.items():
                if name != "natural_log_exp_and_others":
                    t[name] = {f for f in fns if f not in (AFt.Exp, AFt.Ln)}
            return t

        bacc.get_activation_tables = _patched_gat
        bacc._mha_act_tables_patched = True

    f32 = mybir.dt.float32
    f32r = mybir.dt.float32r
    f8 = mybir.dt.float8e4
    DR = mybir.MatmulPerfMode.DoubleRow
    AF = mybir.ActivationFunctionType

    nc = bacc.Bacc("TRN2", target_bir_lowering=False, debug=False, num_devices=B)

    # ONE packed input tensor; slices bitcast/viewed per use. f32r typing:
    # transposes run the 1.5-cyc/row f32r PE path and the phase-3 residual
    # identity-matmul can DMA straight from the x1 slice.
    xin = nc.dram_tensor("xin", [N_ROWS, D], f32r, kind="ExternalInput").ap()
    x1_ap = xin[R_X1:R_X1 + S, :]
    x2_ap = xin[R_X2:R_X2 + S, :]
    x3_ap = xin[R_X3:R_X3 + S, :]
    wq_ap = xin[R_WQ:R_WQ + D, :]
    wk_ap = xin[R_WK:R_WK + D, :]
    wo_ap = xin[R_WO:R_WO + D, :]
    bq_ap = xin[R_BQ:R_BQ + 1, :]
    bk_ap = xin[R_BK:R_BK + 1, :]
    bo_ap = xin[R_BO:R_BO + 1, :]
    gamma_ap = xin[R_GAMMA:R_GAMMA + 1, :].bitcast(f32)
    beta_ap = xin[R_BETA:R_BETA + 1, :].bitcast(f32)
    mf_ap = xin[R_MF:R_MF + 1, :].bitcast(f32)   # [1, 1024] mask row
    y_ap = nc.dram_tensor("y", [S, D], f32, kind="ExternalOutput").ap()

    with tile.TileContext(nc) as tc:
        with tc.tile_pool(name="persist", bufs=1) as persist:
            smalls = persist.tile([128, 512], f32)
            ident = smalls[:, 0:128]
            ones_p = smalls[:, 128:144]      # [128, 16] of ones
            eps_t = smalls[:, 144:145]
            m_sb = smalls[:, 145:153]        # [128, ST] mask per k-tile
            make_identity(nc, ident)
            nc.vector.memset(ones_p, 1.0)
            nc.vector.memset(eps_t, EPS)
            nc.gpsimd.dma_start(m_sb, mf_ap.rearrange("o (t p) -> p (t o)",
                                                      p=128))
            ones_f = persist.tile([1, 512], f32)
            nc.vector.memset(ones_f[:], 1.0)
            ones_r = persist.tile([1, 512], f32r)
            nc.vector.tensor_copy(ones_r[:], ones_f[:])
            ident_r = persist.tile([128, 128], f32r)
            nc.vector.tensor_copy(ident_r[:], ident)
            ident8 = persist.tile([128, 128], f8)
            nc.vector.tensor_copy(ident8[:], ident)
            bk_sb = persist.tile([1, D], f32r)
            nc.gpsimd.dma_start(bk_sb[:], bk_ap)
            # bq/bk transposed to per-partition columns: bT[:, p] = b[p*128:...]
            bqT = persist.tile([128, DT], f32)
            nc.gpsimd.dma_start(
                bqT[:], bq_ap.bitcast(f32).rearrange("o (di p) -> p (o di)",
                                                     p=128))
            bkT = persist.tile([128, DT], f32)
            nc.gpsimd.dma_start(
                bkT[:], bk_ap.bitcast(f32).rearrange("o (di p) -> p (o di)",
                                                     p=128))
            # first Wo row-block prefetched at kernel start: the main wo_sb
            # staging can only DMA after qkv_pool's space frees, which would
            # stall phase 3's first matmuls
            wo0 = persist.tile([128, D], f32r)
            nc.gpsimd.dma_start(wo0[:], wo_ap[0:128, :])

            # oT outlives the phase-1/2 tensors: allocate below them
            oT_pool = tc.alloc_tile_pool(name="oTp", bufs=1)
            oT = oT_pool.tile([128, DT * S], f32r)
            # live through phases 1-2, released before phase 3
            qkv_pool = tc.alloc_tile_pool(name="qkv", bufs=1)
            # x^T tiles in fp8e4: feeds the DoubleRow projection matmuls
            # (2 k-tiles fused per instruction at 0.5 cyc/row => 4x fewer
            # PE cycles than the f32r path); quantization error ~0.15% RMS
            # after the K=1024 contraction, well inside the 2e-2 gate.
            x1T = qkv_pool.tile([128, DT * S], f8)
            x2T = qkv_pool.tile([128, DT * S], f8)
            vaug = qkv_pool.tile([128, ST * H * VW], f32r)  # k-tile t at t*H*VW

            # ------- phase A: transposes; v-projection (mask-augmented) -------
            with tc.tile_pool(name="pA_x3", bufs=1) as x3_pool, \
                 tc.tile_pool(name="pA_w", bufs=6) as w_pool, \
                 tc.tile_pool(name="pA_stage", bufs=10) as stage:

                pA_ps = tc.alloc_tile_pool(name="pA_ps", bufs=8, space="PSUM")

                def transpose_in(x_ap, xT):
                    # xT layout [128, DT*S]: d-tile dt at cols [dt*S + s].
                    # Transposes stay f32r (HW fp8 transposes need a
                    # stride-2 output AP); the fp8 conversion rides the
                    # existing PSUM->SBUF copy as an ACT output-dtype cast.
                    xT3 = xT[:].rearrange("p (d s) -> p d s", s=S)
                    for st in range(ST):
                        for half in range(2):
                            xs = stage.tile([128, 512], f32r, name="xs", tag="xs")
                            nc.sync.dma_start(
                                xs[:], x_ap[st * 128:(st + 1) * 128,
                                            half * 512:(half + 1) * 512])
                            tp = pA_ps.tile([128, 512], f32r, name="tp",
                                            tag="ps512")
                            for j in range(4):
                                nc.tensor.transpose(
                                    tp[:, j * 128:(j + 1) * 128],
                                    xs[:, j * 128:(j + 1) * 128], ident_r[:])
                            dst = xT3[:, half * 4:half * 4 + 4,
                                      st * 128:(st + 1) * 128]
                            nc.scalar.copy(dst, tp[:].rearrange(
                                "p (b c) -> p b c", b=4))

                def v_proj_half(x3T, c):
                    # v natural [S, D] + augmentation with the mask.
                    # fp8 DoubleRow: each matmul contracts TWO 128-row
                    # d-tiles (lhsT [128,2,128] / rhs [128,2,512]) at 0.5
                    # cyc/row -> 4 accumulation steps instead of 8, each
                    # half the cycles.
                    x3T3 = x3T[:].rearrange("p (d s) -> p d s", s=S)
                    pss = [pA_ps.tile([128, 512], f32, name=f"vp{i}",
                                      tag="ps512") for i in range(ST)]
                    for dj in range(DT // 2):
                        wst = w_pool.tile([128, 1024], f32,
                                          name="wstv", tag="wd")
                        nc.sync.dma_start(
                            wst[:].rearrange("p (t m) -> p t m", t=2),
                            wk_ap.bitcast(f32)[
                                dj * 256:(dj + 1) * 256,
                                c * 512:(c + 1) * 512].rearrange(
                                    "(t p) m -> p t m", t=2))
                        wd8 = w_pool.tile([128, 1024], f8,
                                          name="wdv8", tag="wd8")
                        nc.vector.tensor_copy(wd8[:], wst[:])
                        for st in range(ST):
                            nc.tensor.matmul(
                                pss[st][:],
                                x3T3[:, 2 * dj:2 * dj + 2,
                                     st * 128:(st + 1) * 128],
                                wd8[:].rearrange("p (t m) -> p t m", t=2),
                                start=(dj == 0), stop=False,
                                perf_mode=DR)
                    for st in range(ST):
                        nc.tensor.matmul(
                            pss[st][:], ones_r[:, 0:128],
                            bk_sb[:, c * 512:(c + 1) * 512],
                            start=False, stop=True)
                        va = vaug[:, st * H * VW:(st + 1) * H * VW].rearrange(
                            "p (h e) -> p h e", e=VW)
                        nc.vector.tensor_scalar_mul(
                            va[:, 8 * c:8 * (c + 1), 0:DK],
                            pss[st][:].rearrange("p (h e) -> p h e", e=DK),
                            m_sb[:, st:st + 1])
                        if c == 0:
                            nc.vector.tensor_scalar_mul(
                                va[:, :, DK:VW],
                                ones_p.rearrange("p (h e) -> p h e", e=1),
                                m_sb[:, st:st + 1])

                x3T = x3_pool.tile([128, DT * S], f8)
                transpose_in(x3_ap, x3T)
                v_proj_half(x3T, 0)
                transpose_in(x1_ap, x1T)
                v_proj_half(x3T, 1)
                transpose_in(x2_ap, x2T)
                pA_ps.release()

            # --- phase B: per-pair q/k projection pipelined with attention ---
            with tc.tile_pool(name="pB_qk", bufs=2) as qk_pool, \
                 tc.tile_pool(name="pB_w", bufs=8) as w2_pool, \
                 tc.tile_pool(name="pB_P", bufs=4) as P_pool, \
                 tc.tile_pool(name="pB_scr", bufs=2) as scr_pool, \
                 tc.tile_pool(name="pB_pps", bufs=2, space="PSUM") as proj_ps, \
                 tc.tile_pool(name="pB_sps", bufs=2, space="PSUM") as s_ps, \
                 tc.tile_pool(name="pB_ops", bufs=2, space="PSUM") as o_ps:

                def proj_pair_gen(p, w_ap_, bT, xT, out):
                    # out[r, s] = sum_dj (W pair as lhsT) @ xT[dj pair] + b
                    # generator: yields after each dj so the caller can
                    # interleave these into the attention PE stream.
                    # fp8 DoubleRow: lhsT [128,2,128] (two d-tiles of W),
                    # rhs [128,2,512] (matching xT d-tiles, strided AP) ->
                    # 4 accumulation steps at 256 cyc instead of 8 at 512.
                    xT3 = xT[:].rearrange("p (d s) -> p d s", s=S)
                    pps = [proj_ps.tile([128, 512], f32, name=f"pp{c}",
                                        tag="pp") for c in range(NC)]
                    for dj in range(DT // 2):
                        wst = w2_pool.tile([128, 256], f32,
                                           name="wst2", tag="wst2")
                        nc.sync.dma_start(
                            wst[:].rearrange("q (t m) -> q t m", t=2),
                            w_ap_.bitcast(f32)[
                                dj * 256:(dj + 1) * 256,
                                p * 128:(p + 1) * 128].rearrange(
                                    "(t q) m -> q t m", t=2))
                        wd8 = w2_pool.tile([128, 256], f8,
                                           name="wd28", tag="wd28")
                        nc.vector.tensor_copy(wd8[:], wst[:])
                        for c in range(NC):
                            nc.tensor.matmul(
                                pps[c][:],
                                wd8[:].rearrange("q (t m) -> q t m", t=2),
                                xT3[:, 2 * dj:2 * dj + 2,
                                    c * 512:(c + 1) * 512],
                                start=(dj == 0), stop=(dj == DT // 2 - 1),
                                perf_mode=DR)
                        yield
                    for c in range(NC):
                        # bias folded into the PSUM->SBUF copy (per-partition
                        # scalar add); no bias matmul needed
                        nc.vector.tensor_scalar(
                            out[:, c * 512:(c + 1) * 512], pps[c][:],
                            bT[:, p:p + 1], None,
                            op0=mybir.AluOpType.add)
                        yield

                def proj_pair(p):
                    q_t = qk_pool.tile([128, S], f32r, name=f"q{p}", tag="q")
                    k_t = qk_pool.tile([128, S], f32r, name=f"k{p}", tag="k")
                    gq = proj_pair_gen(p, wq_ap, bqT, x1T, q_t)
                    gk = proj_pair_gen(p, wk_ap, bkT, x2T, k_t)
                    return q_t, k_t, gq, gk

                def drain_gen(g, n=1000):
                    for _ in range(n):
                        try:
                            next(g)
                        except StopIteration:
                            return

                qTp, kTp, gq, gk = proj_pair(0)
                drain_gen(gq)
                drain_gen(gk)
                for p in range(NP):
                    # next pair's projections, interleaved into this pair's
                    # attention loop (PE slack absorbs them; ACT stays hot)
                    if p + 1 < NP:
                        qTn, kTn, gq, gk = proj_pair(p + 1)
                    else:
                        qTn = kTn = gq = gk = None
                    for c in range(NC):
                        oaugA = o_ps.tile([VW, 512], f32, name="oaugA", tag="oaug")
                        oaugB = o_ps.tile([VW, 512], f32, name="oaugB", tag="oaug")
                        for kt in range(ST):
                            sc = s_ps.tile([128, 1024], f32, name="sc", tag="sc")
                            nc.tensor.matmul(
                                sc[:, 0:512],
                                kTp[0:64, kt * 128:(kt + 1) * 128],
                                qTp[0:64, c * 512:(c + 1) * 512],
                                start=True, stop=True)
                            nc.tensor.matmul(
                                sc[:, 512:1024],
                                kTp[64:128, kt * 128:(kt + 1) * 128],
                                qTp[64:128, c * 512:(c + 1) * 512],
                                start=True, stop=True)
                            Pt = P_pool.tile([128, 1024], f32r, name="Pt", tag="Pt")
                            nc.scalar.activation(Pt[:], sc[:], AF.Exp,
                                                 scale=1.0 / float(np.sqrt(DK)))
                            base = kt * H * VW
                            nc.tensor.matmul(
                                oaugA[:],
                                vaug[:, base + 2 * p * VW:base + (2 * p + 1) * VW],
                                Pt[:, 0:512],
                                start=(kt == 0), stop=(kt == ST - 1))
                            nc.tensor.matmul(
                                oaugB[:],
                                vaug[:, base + (2 * p + 1) * VW:
                                     base + (2 * p + 2) * VW],
                                Pt[:, 512:1024],
                                start=(kt == 0), stop=(kt == ST - 1))
                            if gq is not None:
                                n = 1 if (kt % 2 == 0 or
                                          (c == 1 and kt in (1, 3))) else 0
                                drain_gen(gq, n)
                                drain_gen(gk, n)
                        # epilogue, both heads batched: stash O rows to free
                        # the banks, ln both denominators, one exp + one
                        # partition-broadcast for the pair
                        stash = scr_pool.tile([64, 1024], f32,
                                              name="stash", tag="stash")
                        nc.vector.tensor_copy(stash[:, 0:512], oaugA[0:64, :])
                        nc.vector.tensor_copy(stash[:, 512:1024], oaugB[0:64, :])
                        # 1/denominator on DVE (reciprocal_approx_fast is a
                        # single custom op, ~51 ULP): keeps the whole epilogue
                        # off ACT, which the exp stream saturates on HW
                        rec = scr_pool.tile([1, 1024], f32, name="rec", tag="rec")
                        nc.vector.tensor_copy(rec[:, 0:512], oaugA[64:65, :])
                        nc.vector.tensor_copy(rec[:, 512:1024], oaugB[64:65, :])
                        nc.vector.reciprocal_approx_fast(rec[:], rec[:])
                        rbc = scr_pool.tile([64, 1024], f32, name="rbc", tag="rbc")
                        nc.gpsimd.partition_broadcast(rbc[:], rec[:])
                        for h_loc in range(2):
                            nc.vector.tensor_mul(
                                oT[h_loc * 64:(h_loc + 1) * 64,
                                   p * S + c * 512:p * S + (c + 1) * 512],
                                stash[:, h_loc * 512:(h_loc + 1) * 512],
                                rbc[:, h_loc * 512:(h_loc + 1) * 512])
                    if gq is not None:
                        drain_gen(gq)
                        drain_gen(gk)
                        qTp, kTp = qTn, kTn
            qkv_pool.release()

            # ---------------- phase 3: out-proj + residual + LayerNorm --------
            with tc.tile_pool(name="p3_w", bufs=1) as w3_pool, \
                 tc.tile_pool(name="p3_stage", bufs=3) as stage3, \
                 tc.tile_pool(name="p3_t", bufs=4) as t_pool, \
                 tc.tile_pool(name="p3_ln", bufs=8) as ln_pool, \
                 tc.tile_pool(name="p3_ps", bufs=4, space="PSUM") as ps3:
                wo_sb = w3_pool.tile([128, (DT - 1) * D], f32r)
                for dt in range(1, DT):
                    nc.sync.dma_start(wo_sb[:, (dt - 1) * D:dt * D],
                                      wo_ap[dt * 128:(dt + 1) * 128, :])
                bo_sb = w3_pool.tile([1, D], f32r)
                nc.sync.dma_start(bo_sb[:], bo_ap)
                gamma_bc = w3_pool.tile([128, D], f32)
                nc.gpsimd.dma_start(gamma_bc[:], gamma_ap.partition_broadcast(128))
                beta_bc = w3_pool.tile([128, D], f32)
                nc.gpsimd.dma_start(beta_bc[:], beta_ap.partition_broadcast(128))
                for qt in range(ST):
                    ps = ps3.tile([128, 1024], f32, name="ps", tag="ps3")
                    xres = stage3.tile([128, D], f32r, name="xres", tag="xres")
                    nc.sync.dma_start(xres[:],
                                      x1_ap[qt * 128:(qt + 1) * 128, :])
                    for di in range(DT):
                        wsrc = (wo0[:] if di == 0 else
                                wo_sb[:, (di - 1) * D:di * D])
                        for c in range(NC):
                            nc.tensor.matmul(
                                ps[:, c * 512:(c + 1) * 512],
                                oT[:, di * S + qt * 128:di * S + (qt + 1) * 128],
                                wsrc[:, c * 512:(c + 1) * 512],
                                start=(di == 0), stop=False)
                    # bo bias and the x1 residual ride the accumulation as
                    # K=1 / identity matmuls (keeps the whole z off DVE)
                    for c in range(NC):
                        nc.tensor.matmul(
                            ps[:, c * 512:(c + 1) * 512], ones_r[:, 0:128],
                            bo_sb[:, c * 512:(c + 1) * 512],
                            start=False, stop=False)
                        nc.tensor.matmul(
                            ps[:, c * 512:(c + 1) * 512], ident_r[:],
                            xres[:, c * 512:(c + 1) * 512],
                            start=False, stop=True)
                    stats = ln_pool.tile([128, NC, 6], f32, name="stats", tag="st")
                    for c in range(NC):
                        nc.vector.bn_stats(stats[:, c, :],
                                           ps[:, c * 512:(c + 1) * 512])
                    mv = ln_pool.tile([128, 2], f32, name="mv", tag="mv")
                    nc.vector.bn_aggr(mv[:], stats[:])
                    # rstd = exp(-0.5*ln(var+eps)): stays in the pinned
                    # exp/ln table set (a Sqrt would force a table reload)
                    rstd = ln_pool.tile([128, 1], f32, name="rstd", tag="rstd")
                    nc.scalar.activation(rstd[:], mv[:, 1:2], AF.Ln, bias=eps_t)
                    nc.scalar.activation(rstd[:], rstd[:], AF.Exp, scale=-0.5)
                    # normalize on ACT (idle in phase 3): (z-mu)*rstd =
                    # Copy(z*rstd + (-mu*rstd)) with per-partition scale/bias
                    nmu = ln_pool.tile([128, 1], f32, name="nmu", tag="nmu")
                    nc.vector.tensor_scalar(
                        nmu[:], rstd[:], mv[:, 0:1], -1.0,
                        op0=mybir.AluOpType.mult,
                        op1=mybir.AluOpType.mult)
                    # per-half chains (DVE half 0, gpsimd half 1): half 0's
                    # store fires while half 1 is still in its affine
                    t2 = t_pool.tile([128, D], f32, name="t2", tag="t2")
                    t3 = t_pool.tile([128, D], f32, name="t3", tag="t3")
                    t4 = t_pool.tile([128, D], f32, name="t4", tag="t4")
                    for h, eng in ((0, nc.vector), (1, nc.gpsimd)):
                        cs = slice(h * 512, (h + 1) * 512)
                        nc.scalar.activation(t2[:, cs], ps[:, cs], AF.Identity,
                                             scale=rstd[:], bias=nmu[:])
                        eng.tensor_mul(t3[:, cs], t2[:, cs], gamma_bc[:, cs])
                        eng.tensor_add(t4[:, cs], t3[:, cs], beta_bc[:, cs])
                        nc.sync.dma_start(
                            y_ap[qt * 128:(qt + 1) * 128, cs], t4[:, cs])
            oT_pool.release()

    nc.compile()
    return nc


def _get_built():
    global _BUILT
    if _BUILT is None:
        _BUILT = _build()
    return _BUILT


def pack_input(b, x1, x2, x3, mf, Wq, Wk, Wo, bq, bk, bo, gamma, beta):
    """Build the packed [6150, 1024] f32 input for batch element b."""
    xin = np.empty((N_ROWS, D), np.float32)
    xin[R_X1:R_X1 + S] = x1[b]
    xin[R_X2:R_X2 + S] = x2[b]
    xin[R_X3:R_X3 + S] = x3[b]
    xin[R_WQ:R_WQ + D] = Wq
    xin[R_WK:R_WK + D] = Wk
    xin[R_WO:R_WO + D] = Wo
    xin[R_MF] = mf[b, 0, :]
    xin[R_BQ] = bq
    xin[R_BK] = bk
    xin[R_BO] = bo
    xin[R_GAMMA] = gamma
    xin[R_BETA] = beta
    return xin


def kernel(x1, x2, x3, mask, Wq, bq, Wk, bk, Wo, bo, gamma, beta):
    from concourse import bass_utils

    nc = _get_built()
    x1 = np.asarray(x1, np.float32)
    x2 = np.asarray(x2, np.float32)
    x3 = np.asarray(x3, np.float32)
    mf = (np.asarray(mask) != 0).astype(np.float32)          # [B, 1, S]
    Wq = np.asarray(Wq, np.float32)
    Wk = np.asarray(Wk, np.float32)
    Wo = np.asarray(Wo, np.float32)
    bq = np.asarray(bq, np.float32).reshape(D)
    bk = np.asarray(bk, np.float32).reshape(D)
    bo = np.asarray(bo, np.float32).reshape(D)
    gamma = np.asarray(gamma, np.float32).reshape(D)
    beta = np.asarray(beta, np.float32).reshape(D)
    in_maps = [
        {"xin": pack_input(b, x1, x2, x3, mf, Wq, Wk, Wo,
                           bq, bk, bo, gamma, beta)}
        for b in range(B)
    ]
    res = bass_utils.run_bass_kernel_spmd(nc, in_maps, core_ids=list(range(B)))
    return np.stack([res.results[b]["y"] for b in range(B)])


# revision 18
# speedup vs baseline: 10.8256x; 1.0700x over previous
"""Trainium2 Bass kernel for nn_MultiHeadAttention (B=8, S=1024, D=1024, H=16).

Sharding: data-parallel over batch — 8 NeuronCores, one batch element each;
weights replicated. No collectives needed.

IO packing: the per-call dispatch overhead through the axon tunnel scales
with the number of IO tensors (~50us/tensor/call) and IO bytes, and
dominates the pipelined per-call wall time (device body ~0.35ms overlaps
the dispatch pipeline entirely).  All 13 inputs are therefore packed into
ONE [6150, 1024] f32 DRAM tensor per core (x1|x2|x3|wq|wk|wo rows 0-6143,
then mf/bq/bk/bo/gamma/beta one row each), cutting per-call tensor count
from 15 to 3.

Per-core compute plan (all matmul contractions on the partition dim).
The q/k/v projections run as fp8e4m3 DoubleRow matmuls (both operands
fp8, two 128-row d-tiles fused per instruction at 0.5 cyc/row => 4x
fewer PE cycles than f32r); x^T tiles are cast to fp8 in the
transpose PSUM->SBUF copy and weights on DVE after staging.  Scores,
PV, and the out-projection stay f32r (full rate at N=512).  Measured
rel err 9.5e-3 on HW (gate 2e-2; error dominated by fp8 q/k
quantization amplified through exp).  Paired 10-round A/B on HW:
-47 us/call median vs the f32r projections:

  phase A: PE-transpose x3/x1/x2 128x128 blocks (identity matmul) into
           x^T layouts; project v = (x3T as lhsT) @ Wk (natural [S, D]),
           interleaved with the x1/x2 transposes.  The bk bias rides as a
           K=1 accumulating matmul.  The key/pad mask is folded into an
           augmented value matrix vaug = [m * v | m] so masking AND the
           softmax denominator ride the PV matmul for free
           (P*m @ v == P @ (m*v), denom = P @ m).
  phase B: per head-pair p: q^T/k^T projections for pair p+1 are emitted
           as generators interleaved into pair p's attention loop (PE
           slack absorbs them, ACT stays saturated).  bq/bk biases fold
           into the PSUM->SBUF copy as per-partition tensor_scalar adds
           (no bias matmuls).  Scores S^T[k,q] = kT-slice^T @ qT-slice
           (K=64, the two heads auto-row-tile into PE row-groups via
           base_partition 0/64 => concurrent on HW); P^T = exp(S^T/8)
           via one [128,1024] ACT op per k-tile straight out of PSUM (no
           max-subtraction: |scores/8| <= ~7 is fp32-safe); O^T_aug[65,
           q] += vaug-slice^T @ P^T accumulated over k-tiles (row 64 =
           softmax denominator); epilogue per (pair, chunk): O rows
           stashed to SBUF to free the banks, 1/denom via DVE
           reciprocal_approx_fast (single custom op, ~51 ULP; keeps the
           epilogue off ACT, which the exp stream saturates on HW; plain
           DVE reciprocal is 8 cyc/elem, ACT Reciprocal/Rsqrt banned),
           partition-broadcast via the gpsimd ucode (SBUF->SBUF, no DRAM
           roundtrip), one [64,1024]-wide normalize pass into oT.  Exp
           and Ln are pinned to one activation-table set; the whole
           kernel uses only Exp/Ln/Identity so the table loads once.
  phase 3: out = (oT as lhsT) @ Wo; the bo bias (K=1 ones matmul) and
           the x1 residual (f32r identity matmul) ride the same PSUM
           accumulation, so LayerNorm stats read PSUM directly
           (bn_stats/bn_aggr on DVE); rstd = exp(-0.5*ln(var+eps)) stays
           in the pinned table set; the normalize (z-mu)*rstd runs on
           ACT as Identity(z*rstd + (-mu*rstd)) with per-partition
           scale/bias APs; gamma/beta halves split between DVE and
           gpsimd so no single engine owns the tail; wo row-block 0 is
           prefetched at kernel start (the main wo staging can only
           start once qkv SBUF frees).
"""
import sys

if "/opt/trn_rl_repo" not in sys.path:
    sys.path.insert(0, "/opt/trn_rl_repo")

import numpy as np

B, S, D, H = 8, 1024, 1024, 16
DK = D // H          # 64
NP = H // 2          # 8 head pairs
ST = S // 128        # 8 s-tiles (also k-tiles)
DT = D // 128        # 8 d-tiles
NC = S // 512        # 2 chunks of 512
VW = DK + 1          # 65: augmented head width
EPS = 1e-5

# packed input layout: row offsets into xin [6150, 1024]
R_X1, R_X2, R_X3 = 0, S, 2 * S
R_WQ, R_WK, R_WO = 3 * S, 3 * S + D, 3 * S + 2 * D
R_MF = 3 * S + 3 * D          # 6144: mask row [1, 1024]
R_BQ, R_BK, R_BO = R_MF + 1, R_MF + 2, R_MF + 3
R_GAMMA, R_BETA = R_MF + 4, R_MF + 5
N_ROWS = R_MF + 6             # 6150

_BUILT = None


def _build():
    import concourse.bass as bass  # noqa: F401
    import concourse.tile as tile
    from concourse import bacc, mybir
    from concourse.masks import make_identity

    # Keep Exp and Ln in one activation-table set: remove them from every
    # other set (set order/indices preserved) so the table-load pass resolves
    # both to natural_log_exp_and_others instead of thrashing 33 reloads.
    AFt = mybir.ActivationFunctionType
    if not getattr(bacc, "_mha_act_tables_patched", False):
        orig_gat = bacc.get_activation_tables

        def _patched_gat(arch):
            t = dict(orig_gat(arch))
            for name, fns in t.items():
                if name != "natural_log_exp_and_others":
                    t[name] = {f for f in fns if f not in (AFt.Exp, AFt.Ln)}
            return t

        bacc.get_activation_tables = _patched_gat
        bacc._mha_act_tables_patched = True

    f32 = mybir.dt.float32
    f32r = mybir.dt.float32r
    f8 = mybir.dt.float8e4
    DR = mybir.MatmulPerfMode.DoubleRow
    AF = mybir.ActivationFunctionType

    nc = bacc.Bacc("TRN2", target_bir_lowering=False, debug=False, num_devices=B)

    # ONE packed input tensor; slices bitcast/viewed per use. f32r typing:
    # transposes run the 1.5-cyc/row f32r PE path and the phase-3 residual
    # identity-matmul can DMA straight from the x1 slice.
    xin = nc.dram_tensor("xin", [N_ROWS, D], f32r, kind="ExternalInput").ap()
    x1_ap = xin[R_X1:R_X1 + S, :]
    x2_ap = xin[R_X2:R_X2 + S, :]
    x3_ap = xin[R_X3:R_X3 + S, :]
    wq_ap = xin[R_WQ:R_WQ + D, :]
    wk_ap = xin[R_WK:R_WK + D, :]
    wo_ap = xin[R_WO:R_WO + D, :]
    bq_ap = xin[R_BQ:R_BQ + 1, :]
    bk_ap = xin[R_BK:R_BK + 1, :]
    bo_ap = xin[R_BO:R_BO + 1, :]
    gamma_ap = xin[R_GAMMA:R_GAMMA + 1, :].bitcast(f32)
    beta_ap = xin[R_BETA:R_BETA + 1, :].bitcast(f32)
    mf_ap = xin[R_MF:R_MF + 1, :].bitcast(f32)   # [1, 1024] mask row
    y_ap = nc.dram_tensor("y", [S, D], f32, kind="ExternalOutput").ap()

    with tile.TileContext(nc) as tc:
        with tc.tile_pool(name="persist", bufs=1) as persist:
            smalls = persist.tile([128, 512], f32)
            ident = smalls[:, 0:128]
            ones_p = smalls[:, 128:144]      # [128, 16] of ones
            eps_t = smalls[:, 144:145]
            m_sb = smalls[:, 145:153]        # [128, ST] mask per k-tile
            nb4 = smalls[:, 153:154]         # exp bias: P = exp(s/8 - 4)
            make_identity(nc, ident)
            nc.vector.memset(ones_p, 1.0)
            nc.vector.memset(eps_t, EPS)
            nc.vector.memset(nb4, -4.0)
            nc.gpsimd.dma_start(m_sb, mf_ap.rearrange("o (t p) -> p (t o)",
                                                      p=128))
            ones_f = persist.tile([1, 512], f32)
            nc.vector.memset(ones_f[:], 1.0)
            ones_r = persist.tile([1, 512], f32r)
            nc.vector.tensor_copy(ones_r[:], ones_f[:])
            ident_r = persist.tile([128, 128], f32r)
            nc.vector.tensor_copy(ident_r[:], ident)
            ident8 = persist.tile([128, 128], f8)
            nc.vector.tensor_copy(ident8[:], ident)
            bk_sb = persist.tile([1, D], f32r)
            nc.gpsimd.dma_start(bk_sb[:], bk_ap)
            # bq/bk transposed to per-partition columns: bT[:, p] = b[p*128:...]
            bqT = persist.tile([128, DT], f32)
            nc.gpsimd.dma_start(
                bqT[:], bq_ap.bitcast(f32).rearrange("o (di p) -> p (o di)",
                                                     p=128))
            bkT = persist.tile([128, DT], f32)
            nc.gpsimd.dma_start(
                bkT[:], bk_ap.bitcast(f32).rearrange("o (di p) -> p (o di)",
                                                     p=128))
            # first Wo row-block prefetched at kernel start: the main wo_sb
            # staging can only DMA after qkv_pool's space frees, which would
            # stall phase 3's first matmuls
            wo0 = persist.tile([128, D], f32r)
            nc.gpsimd.dma_start(wo0[:], wo_ap[0:128, :])

            # oT outlives the phase-1/2 tensors: allocate below them
            oT_pool = tc.alloc_tile_pool(name="oTp", bufs=1)
            oT = oT_pool.tile([128, DT * S], f32r)
            # live through phases 1-2, released before phase 3
            qkv_pool = tc.alloc_tile_pool(name="qkv", bufs=1)
            # x^T tiles in fp8e4: feeds the DoubleRow projection matmuls
            # (2 k-tiles fused per instruction at 0.5 cyc/row => 4x fewer
            # PE cycles than the f32r path); quantization error ~0.15% RMS
            # after the K=1024 contraction, well inside the 2e-2 gate.
            x1T = qkv_pool.tile([128, DT * S], f8)
            x2T = qkv_pool.tile([128, DT * S], f8)
            # vaug in fp8e4: feeds the DoubleRow PV matmul (paired with fp8
            # P).  P is exp(s/8 - 4): the -4 keeps exp <= e^4.4 = 82 inside
            # e4m3's 448 range (raw scores reach +-8.4) and cancels exactly
            # in the softmax division.  Numpy-simulated end-to-end rel err
            # 1.18e-2 vs the 2e-2 gate (sim calibrated to HW at 3 digits).
            vaug = qkv_pool.tile([128, ST * H * VW], f8)    # k-tile t at t*H*VW

            # ------- phase A: transposes; v-projection (mask-augmented) -------
            with tc.tile_pool(name="pA_x3", bufs=1) as x3_pool, \
                 tc.tile_pool(name="pA_w", bufs=6) as w_pool, \
                 tc.tile_pool(name="pA_stage", bufs=10) as stage:

                pA_ps = tc.alloc_tile_pool(name="pA_ps", bufs=8, space="PSUM")

                def transpose_in(x_ap, xT):
                    # xT layout [128, DT*S]: d-tile dt at cols [dt*S + s].
                    # Transposes stay f32r (HW fp8 transposes need a
                    # stride-2 output AP); the fp8 conversion rides the
                    # existing PSUM->SBUF copy as an ACT output-dtype cast.
                    xT3 = xT[:].rearrange("p (d s) -> p d s", s=S)
                    for st in range(ST):
                        for half in range(2):
                            xs = stage.tile([128, 512], f32r, name="xs", tag="xs")
                            nc.sync.dma_start(
                                xs[:], x_ap[st * 128:(st + 1) * 128,
                                            half * 512:(half + 1) * 512])
                            tp = pA_ps.tile([128, 512], f32r, name="tp",
                                            tag="ps512")
                            for j in range(4):
                                nc.tensor.transpose(
                                    tp[:, j * 128:(j + 1) * 128],
                                    xs[:, j * 128:(j + 1) * 128], ident_r[:])
                            dst = xT3[:, half * 4:half * 4 + 4,
                                      st * 128:(st + 1) * 128]
                            nc.scalar.copy(dst, tp[:].rearrange(
                                "p (b c) -> p b c", b=4))

                def v_proj_half(x3T, c):
                    # v natural [S, D] + augmentation with the mask.
                    # fp8 DoubleRow: each matmul contracts TWO 128-row
                    # d-tiles (lhsT [128,2,128] / rhs [128,2,512]) at 0.5
                    # cyc/row -> 4 accumulation steps instead of 8, each
                    # half the cycles.
                    x3T3 = x3T[:].rearrange("p (d s) -> p d s", s=S)
                    pss = [pA_ps.tile([128, 512], f32, name=f"vp{i}",
                                      tag="ps512") for i in range(ST)]
                    for dj in range(DT // 2):
                        wst = w_pool.tile([128, 1024], f32,
                                          name="wstv", tag="wd")
                        nc.sync.dma_start(
                            wst[:].rearrange("p (t m) -> p t m", t=2),
                            wk_ap.bitcast(f32)[
                                dj * 256:(dj + 1) * 256,
                                c * 512:(c + 1) * 512].rearrange(
                                    "(t p) m -> p t m", t=2))
                        wd8 = w_pool.tile([128, 1024], f8,
                                          name="wdv8", tag="wd8")
                        nc.vector.tensor_copy(wd8[:], wst[:])
                        for st in range(ST):
                            nc.tensor.matmul(
                                pss[st][:],
                                x3T3[:, 2 * dj:2 * dj + 2,
                                     st * 128:(st + 1) * 128],
                                wd8[:].rearrange("p (t m) -> p t m", t=2),
                                start=(dj == 0), stop=False,
                                perf_mode=DR)
                    for st in range(ST):
                        nc.tensor.matmul(
                            pss[st][:], ones_r[:, 0:128],
                            bk_sb[:, c * 512:(c + 1) * 512],
                            start=False, stop=True)
                        va = vaug[:, st * H * VW:(st + 1) * H * VW].rearrange(
                            "p (h e) -> p h e", e=VW)
                        nc.vector.tensor_scalar_mul(
                            va[:, 8 * c:8 * (c + 1), 0:DK],
                            pss[st][:].rearrange("p (h e) -> p h e", e=DK),
                            m_sb[:, st:st + 1])
                        if c == 0:
                            nc.vector.tensor_scalar_mul(
                                va[:, :, DK:VW],
                                ones_p.rearrange("p (h e) -> p h e", e=1),
                                m_sb[:, st:st + 1])

                x3T = x3_pool.tile([128, DT * S], f8)
                transpose_in(x3_ap, x3T)
                v_proj_half(x3T, 0)
                transpose_in(x1_ap, x1T)
                v_proj_half(x3T, 1)
                transpose_in(x2_ap, x2T)
                pA_ps.release()

            # --- phase B: per-pair q/k projection pipelined with attention ---
            with tc.tile_pool(name="pB_qk", bufs=2) as qk_pool, \
                 tc.tile_pool(name="pB_w", bufs=8) as w2_pool, \
                 tc.tile_pool(name="pB_P", bufs=4) as P_pool, \
                 tc.tile_pool(name="pB_scr", bufs=2) as scr_pool, \
                 tc.tile_pool(name="pB_pps", bufs=2, space="PSUM") as proj_ps, \
                 tc.tile_pool(name="pB_sps", bufs=2, space="PSUM") as s_ps, \
                 tc.tile_pool(name="pB_ops", bufs=2, space="PSUM") as o_ps:

                def proj_pair_gen(p, w_ap_, bT, xT, out):
                    # out[r, s] = sum_dj (W pair as lhsT) @ xT[dj pair] + b
                    # generator: yields after each dj so the caller can
                    # interleave these into the attention PE stream.
                    # fp8 DoubleRow: lhsT [128,2,128] (two d-tiles of W),
                    # rhs [128,2,512] (matching xT d-tiles, strided AP) ->
                    # 4 accumulation steps at 256 cyc instead of 8 at 512.
                    xT3 = xT[:].rearrange("p (d s) -> p d s", s=S)
                    pps = [proj_ps.tile([128, 512], f32, name=f"pp{c}",
                                        tag="pp") for c in range(NC)]
                    for dj in range(DT // 2):
                        wst = w2_pool.tile([128, 256], f32,
                                           name="wst2", tag="wst2")
                        nc.sync.dma_start(
                            wst[:].rearrange("q (t m) -> q t m", t=2),
                            w_ap_.bitcast(f32)[
                                dj * 256:(dj + 1) * 256,
                                p * 128:(p + 1) * 128].rearrange(
                                    "(t q) m -> q t m", t=2))
                        wd8 = w2_pool.tile([128, 256], f8,
                                           name="wd28", tag="wd28")
                        nc.vector.tensor_copy(wd8[:], wst[:])
                        for c in range(NC):
                            nc.tensor.matmul(
                                pps[c][:],
                                wd8[:].rearrange("q (t m) -> q t m", t=2),
                                xT3[:, 2 * dj:2 * dj + 2,
                                    c * 512:(c + 1) * 512],
                                start=(dj == 0), stop=(dj == DT // 2 - 1),
                                perf_mode=DR)
                        yield
                    for c in range(NC):
                        # bias folded into the PSUM->SBUF copy (per-partition
                        # scalar add); no bias matmul needed
                        nc.vector.tensor_scalar(
                            out[:, c * 512:(c + 1) * 512], pps[c][:],
                            bT[:, p:p + 1], None,
                            op0=mybir.AluOpType.add)
                        yield

                def proj_pair(p):
                    q_t = qk_pool.tile([128, S], f32r, name=f"q{p}", tag="q")
                    k_t = qk_pool.tile([128, S], f32r, name=f"k{p}", tag="k")
                    gq = proj_pair_gen(p, wq_ap, bqT, x1T, q_t)
                    gk = proj_pair_gen(p, wk_ap, bkT, x2T, k_t)
                    return q_t, k_t, gq, gk

                def drain_gen(g, n=1000):
                    for _ in range(n):
                        try:
                            next(g)
                        except StopIteration:
                            return

                qTp, kTp, gq, gk = proj_pair(0)
                drain_gen(gq)
                drain_gen(gk)
                for p in range(NP):
                    # next pair's projections, interleaved into this pair's
                    # attention loop (PE slack absorbs them; ACT stays hot)
                    if p + 1 < NP:
                        qTn, kTn, gq, gk = proj_pair(p + 1)
                    else:
                        qTn = kTn = gq = gk = None
                    vaug4 = vaug[:].rearrange("p (k h e) -> p k h e",
                                              h=H, e=VW)
                    for c in range(NC):
                        oaugA = o_ps.tile([VW, 512], f32, name="oaugA", tag="oaug")
                        oaugB = o_ps.tile([VW, 512], f32, name="oaugB", tag="oaug")
                        Pt = None
                        for kt in range(ST):
                            sc = s_ps.tile([128, 1024], f32, name="sc", tag="sc")
                            nc.tensor.matmul(
                                sc[:, 0:512],
                                kTp[0:64, kt * 128:(kt + 1) * 128],
                                qTp[0:64, c * 512:(c + 1) * 512],
                                start=True, stop=True)
                            nc.tensor.matmul(
                                sc[:, 512:1024],
                                kTp[64:128, kt * 128:(kt + 1) * 128],
                                qTp[64:128, c * 512:(c + 1) * 512],
                                start=True, stop=True)
                            # P in fp8 paired over two k-tiles: [t, head, q]
                            if kt % 2 == 0:
                                Pt = P_pool.tile([128, 2048], f8,
                                                 name="Pt", tag="Pt")
                            nc.scalar.activation(
                                Pt[:, (kt % 2) * 1024:(kt % 2 + 1) * 1024],
                                sc[:], AF.Exp,
                                scale=1.0 / float(np.sqrt(DK)), bias=nb4)
                            if kt % 2 == 1:
                                # PV as fp8 DoubleRow: both k-tiles of the
                                # pair contracted in one instruction per
                                # head (lhsT [128,2,65] / rhs [128,2,512])
                                kj = kt // 2
                                Pt4 = Pt[:].rearrange(
                                    "p (t h m) -> p t h m", t=2, h=2)
                                nc.tensor.matmul(
                                    oaugA[:],
                                    vaug4[:, 2 * kj:2 * kj + 2, 2 * p, :],
                                    Pt4[:, :, 0, :],
                                    start=(kj == 0), stop=(kj == ST // 2 - 1),
                                    perf_mode=DR)
                                nc.tensor.matmul(
                                    oaugB[:],
                                    vaug4[:, 2 * kj:2 * kj + 2, 2 * p + 1, :],
                                    Pt4[:, :, 1, :],
                                    start=(kj == 0), stop=(kj == ST // 2 - 1),
                                    perf_mode=DR)
                            if gq is not None:
                                n = 1 if (kt % 2 == 0 or
                                          (c == 1 and kt in (1, 3))) else 0
                                drain_gen(gq, n)
                                drain_gen(gk, n)
                        # epilogue, both heads batched: stash O rows to free
                        # the banks, ln both denominators, one exp + one
                        # partition-broadcast for the pair
                        stash = scr_pool.tile([64, 1024], f32,
                                              name="stash", tag="stash")
                        nc.vector.tensor_copy(stash[:, 0:512], oaugA[0:64, :])
                        nc.vector.tensor_copy(stash[:, 512:1024], oaugB[0:64, :])
                        # 1/denominator on DVE (reciprocal_approx_fast is a
                        # single custom op, ~51 ULP): keeps the whole epilogue
                        # off ACT, which the exp stream saturates on HW
                        rec = scr_pool.tile([1, 1024], f32, name="rec", tag="rec")
                        nc.vector.tensor_copy(rec[:, 0:512], oaugA[64:65, :])
                        nc.vector.tensor_copy(rec[:, 512:1024], oaugB[64:65, :])
                        nc.vector.reciprocal_approx_fast(rec[:], rec[:])
                        rbc = scr_pool.tile([64, 1024], f32, name="rbc", tag="rbc")
                        nc.gpsimd.partition_broadcast(rbc[:], rec[:])
                        for h_loc in range(2):
                            nc.vector.tensor_mul(
                                oT[h_loc * 64:(h_loc + 1) * 64,
                                   p * S + c * 512:p * S + (c + 1) * 512],
                                stash[:, h_loc * 512:(h_loc + 1) * 512],
                                rbc[:, h_loc * 512:(h_loc + 1) * 512])
                    if gq is not None:
                        drain_gen(gq)
                        drain_gen(gk)
                        qTp, kTp = qTn, kTn
            qkv_pool.release()

            # ---------------- phase 3: out-proj + residual + LayerNorm --------
            with tc.tile_pool(name="p3_w", bufs=1) as w3_pool, \
                 tc.tile_pool(name="p3_stage", bufs=3) as stage3, \
                 tc.tile_pool(name="p3_t", bufs=4) as t_pool, \
                 tc.tile_pool(name="p3_ln", bufs=8) as ln_pool, \
                 tc.tile_pool(name="p3_ps", bufs=4, space="PSUM") as ps3:
                wo_sb = w3_pool.tile([128, (DT - 1) * D], f32r)
                for dt in range(1, DT):
                    nc.sync.dma_start(wo_sb[:, (dt - 1) * D:dt * D],
                                      wo_ap[dt * 128:(dt + 1) * 128, :])
                bo_sb = w3_pool.tile([1, D], f32r)
                nc.sync.dma_start(bo_sb[:], bo_ap)
                gamma_bc = w3_pool.tile([128, D], f32)
                nc.gpsimd.dma_start(gamma_bc[:], gamma_ap.partition_broadcast(128))
                beta_bc = w3_pool.tile([128, D], f32)
                nc.gpsimd.dma_start(beta_bc[:], beta_ap.partition_broadcast(128))
                for qt in range(ST):
                    ps = ps3.tile([128, 1024], f32, name="ps", tag="ps3")
                    xres = stage3.tile([128, D], f32r, name="xres", tag="xres")
                    nc.sync.dma_start(xres[:],
                                      x1_ap[qt * 128:(qt + 1) * 128, :])
                    for di in range(DT):
                        wsrc = (wo0[:] if di == 0 else
                                wo_sb[:, (di - 1) * D:di * D])
                        for c in range(NC):
                            nc.tensor.matmul(
                                ps[:, c * 512:(c + 1) * 512],
                                oT[:, di * S + qt * 128:di * S + (qt + 1) * 128],
                                wsrc[:, c * 512:(c + 1) * 512],
                                start=(di == 0), stop=False)
                    # bo bias and the x1 residual ride the accumulation as
                    # K=1 / identity matmuls (keeps the whole z off DVE)
                    for c in range(NC):
                        nc.tensor.matmul(
                            ps[:, c * 512:(c + 1) * 512], ones_r[:, 0:128],
                            bo_sb[:, c * 512:(c + 1) * 512],
                            start=False, stop=False)
                        nc.tensor.matmul(
                            ps[:, c * 512:(c + 1) * 512], ident_r[:],
                            xres[:, c * 512:(c + 1) * 512],
                            start=False, stop=True)
                    stats = ln_pool.tile([128, NC, 6], f32, name="stats", tag="st")
                    for c in range(NC):
                        nc.vector.bn_stats(stats[:, c, :],
                                           ps[:, c * 512:(c + 1) * 512])
                    mv = ln_pool.tile([128, 2], f32, name="mv", tag="mv")
                    nc.vector.bn_aggr(mv[:], stats[:])
                    # rstd = exp(-0.5*ln(var+eps)): stays in the pinned
                    # exp/ln table set (a Sqrt would force a table reload)
                    rstd = ln_pool.tile([128, 1], f32, name="rstd", tag="rstd")
                    nc.scalar.activation(rstd[:], mv[:, 1:2], AF.Ln, bias=eps_t)
                    nc.scalar.activation(rstd[:], rstd[:], AF.Exp, scale=-0.5)
                    # normalize on ACT (idle in phase 3): (z-mu)*rstd =
                    # Copy(z*rstd + (-mu*rstd)) with per-partition scale/bias
                    nmu = ln_pool.tile([128, 1], f32, name="nmu", tag="nmu")
                    nc.vector.tensor_scalar(
                        nmu[:], rstd[:], mv[:, 0:1], -1.0,
                        op0=mybir.AluOpType.mult,
                        op1=mybir.AluOpType.mult)
                    # per-half chains (DVE half 0, gpsimd half 1): half 0's
                    # store fires while half 1 is still in its affine
                    t2 = t_pool.tile([128, D], f32, name="t2", tag="t2")
                    t3 = t_pool.tile([128, D], f32, name="t3", tag="t3")
                    t4 = t_pool.tile([128, D], f32, name="t4", tag="t4")
                    for h, eng in ((0, nc.vector), (1, nc.gpsimd)):
                        cs = slice(h * 512, (h + 1) * 512)
                        nc.scalar.activation(t2[:, cs], ps[:, cs], AF.Identity,
                                             scale=rstd[:], bias=nmu[:])
                        eng.tensor_mul(t3[:, cs], t2[:, cs], gamma_bc[:, cs])
                        eng.tensor_add(t4[:, cs], t3[:, cs], beta_bc[:, cs])
                        nc.sync.dma_start(
                            y_ap[qt * 128:(qt + 1) * 128, cs], t4[:, cs])
            oT_pool.release()

    nc.compile()
    return nc


def _get_built():
    global _BUILT
    if _BUILT is None:
        _BUILT = _build()
    return _BUILT


def pack_input(b, x1, x2, x3, mf, Wq, Wk, Wo, bq, bk, bo, gamma, beta):
    """Build the packed [6150, 1024] f32 input for batch element b."""
    xin = np.empty((N_ROWS, D), np.float32)
    xin[R_X1:R_X1 + S] = x1[b]
    xin[R_X2:R_X2 + S] = x2[b]
    xin[R_X3:R_X3 + S] = x3[b]
    xin[R_WQ:R_WQ + D] = Wq
    xin[R_WK:R_WK + D] = Wk
    xin[R_WO:R_WO + D] = Wo
    xin[R_MF] = mf[b, 0, :]
    xin[R_BQ] = bq
    xin[R_BK] = bk
    xin[R_BO] = bo
    xin[R_GAMMA] = gamma
    xin[R_BETA] = beta
    return xin


def kernel(x1, x2, x3, mask, Wq, bq, Wk, bk, Wo, bo, gamma, beta):
    from concourse import bass_utils

    nc = _get_built()
    x1 = np.asarray(x1, np.float32)
    x2 = np.asarray(x2, np.float32)
    x3 = np.asarray(x3, np.float32)
    mf = (np.asarray(mask) != 0).astype(np.float32)          # [B, 1, S]
    Wq = np.asarray(Wq, np.float32)
    Wk = np.asarray(Wk, np.float32)
    Wo = np.asarray(Wo, np.float32)
    bq = np.asarray(bq, np.float32).reshape(D)
    bk = np.asarray(bk, np.float32).reshape(D)
    bo = np.asarray(bo, np.float32).reshape(D)
    gamma = np.asarray(gamma, np.float32).reshape(D)
    beta = np.asarray(beta, np.float32).reshape(D)
    in_maps = [
        {"xin": pack_input(b, x1, x2, x3, mf, Wq, Wk, Wo,
                           bq, bk, bo, gamma, beta)}
        for b in range(B)
    ]
    res = bass_utils.run_bass_kernel_spmd(nc, in_maps, core_ids=list(range(B)))
    return np.stack([res.results[b]["y"] for b in range(B)])
